# revision 16
# baseline (speedup 1.0000x reference)
"""GCN (2-layer, PyG GCNConv semantics) on 8 Trainium2 NeuronCores.

Strategy (dst-sharded message passing):
  out = softmax( A @ relu(A @ (x W1) + b1) @ W2 + b2 ),  A = D^-1/2 (Adj+I) D^-1/2

  - Host: degrees/dinv, self-loops appended as ordinary edges, edges
    partitioned by destination core (6250 dst rows per core), each core's
    dst nodes permuted into 50 load-balanced blocks of 128.  Per-edge
    gather indices (int16) and one-hot segment-sum matrices (bf16, with
    dinv[dst] folded in; bias rows folded in as extra "edges") are
    precomputed on the host and streamed to the device.
  - Phase 0 (on-device, redundant per core): z1 = (dinv*x) @ W1 in bf16,
    stored to local HBM (the layer-1 gather table).
  - Phase 1: per-edge dma_gather of z1 rows (4 SWDGE queues round-robin,
    4096-row pieces); segment-sum via TensorE matmuls h += S^T @ G
    (S = one-hot with dinv[dst]); relu on ScalarE; z2 = dinv * (h @ W2).
  - AllGather of z2 (bf16, rows padded to 128 cols) across the 8 cores
    in two row-slices.
  - Phase 2: per-edge dma_gather of z2 rows (bf16 256B rows), segment-sum
    to output blocks, softmax, DMA out.

kernel(**inputs) -> np.ndarray is self-contained (shapes hardcoded).
"""

import os
import sys
import types

sys.path.insert(0, "/opt/trn_rl_repo")

import numpy as np
import ml_dtypes

from concourse import bass, mybir, bacc, tile
from concourse.bass_utils import run_bass_kernel_spmd

BF16 = ml_dtypes.bfloat16

# ---------------- problem constants (hardcoded) ----------------
N_NODES = 50000
D_IN, D_HID, D_OUT = 512, 256, 64
NCORES = 8
RPC = N_NODES // NCORES          # 6250 dst rows per core
BLK = 128
BPC = 50                         # blocks per core (spare slots for balancing)
RPAD = BPC * BLK                 # 6400
NPAD = ((N_NODES + BLK - 1) // BLK) * BLK   # 50048 (391 node blocks)
NBLOCKS = NPAD // BLK            # 391
SPLIT1 = 24960                   # L1 gather src split (block-aligned, int16-safe)
S0_ROWS = 3200                   # AG slice 0: perm positions [0, 3200) = 25 blocks
S1_ROWS = RPAD - S0_ROWS         # 3200: positions [3200, 6400) = 25 blocks
S0_BLOCKS = S0_ROWS // BLK       # 25
CPR = S0_ROWS + 16               # rows per AG contribution (16 = b2 bias pad)
Z2ROWS = NCORES * CPR            # 25728 rows per z2 table
PIECE = int(os.environ.get("GCN_PIECE", "1024"))   # slots per dma_gather
                                 # (>1024 overflows the SWDGE ring: hangs)
S2CH = 16                        # one-hot chunks per S2 stream DMA piece
NQ = 4                           # SWDGE queues (ucode max)

LAST = {}                        # test harness introspection


def _install_trace_hook():
    try:
        mod = types.ModuleType("antenv.axon_hooks")
        hook = [None]
        mod.set_axon_ntff_profile_hook = lambda h: hook.__setitem__(0, h)
        mod.get_axon_ntff_profile_hook = lambda: hook[0]
        sys.modules["antenv.axon_hooks"] = mod
        import antenv
        antenv.axon_hooks = mod
        from trn_agent_boot.trn_boot import _ntff_profile_via_ctypes
        mod.set_axon_ntff_profile_hook(
            _ntff_profile_via_ctypes("/opt/axon/libaxon_pjrt.so"))
        return True
    except Exception:
        return False


# ---------------- host-side preprocessing ----------------

def _pack_greedy(node_ids, cnts, block_ids, cap):
    """Greedy k-dim balanced packing of node_ids into block_ids (<=128 each).
    cnts: [ndim, RPC] per-node counts. Returns {node: block}."""
    nd = len(cnts)
    nb = len(block_ids)
    tot = sum(c[node_ids] for c in cnts)
    order = node_ids[np.argsort(-tot, kind="stable")]
    sums = np.zeros((nd, nb), dtype=np.float64)
    cnt = np.zeros(nb, dtype=np.int64)
    assign = {}
    big = 1e18
    for i in order:
        score = np.max([(sums[d] + cnts[d][i]) / cap for d in range(nd)], axis=0)
        score = score + (sums.sum(axis=0) + tot[0] * 0) * 1e-7
        score = np.where(cnt < BLK, score, big)
        j = int(np.argmin(score))
        assign[i] = j
        cnt[j] += 1
        for d in range(nd):
            sums[d, j] += cnts[d][i]
    # repair per dim
    members = {j: [i for i, jj in assign.items() if jj == j] for j in range(nb)}
    for d in range(nd):
        for _ in range(2000):
            j = int(np.argmax(sums[d]))
            if sums[d, j] <= cap:
                break
            ms = members[j]
            pos_m = [i for i in ms if cnts[d][i] > 0]
            if not pos_m:
                break
            mv = min(pos_m, key=lambda i: cnts[d][i])
            tgt = np.where(cnt < BLK, sums[d], big)
            tgt[j] = big
            jt = int(np.argmin(tgt))
            if tgt[jt] >= big:
                break
            assign[mv] = jt
            members[j].remove(mv)
            members[jt].append(mv)
            cnt[j] -= 1
            cnt[jt] += 1
            for dd in range(nd):
                sums[dd, j] -= cnts[dd][mv]
                sums[dd, jt] += cnts[dd][mv]
    return assign


def _positions_from_assign(assign, block_ids):
    pos = {}
    slot = {j: 0 for j in block_ids}
    for i in sorted(assign):
        j = assign[i]
        pos[i] = j * BLK + slot[j]
        slot[j] += 1
    return pos


def _pack_blocks(cntA, cntB, cap=1148):
    nodes = np.arange(RPC)
    assign = _pack_greedy(nodes, [cntA, cntB], list(range(BPC)), cap)
    posd = _positions_from_assign(assign, list(range(BPC)))
    pos = np.empty(RPC, dtype=np.int64)
    for i in range(RPC):
        pos[i] = posd[i]
    return pos


def _pack_blocks4(cntA, cntB, cntC, cntD, half0_nodes, cap=1148):
    """Second pass: rebalance within halves on 4 dims."""
    pos = np.empty(RPC, dtype=np.int64)
    all_nodes = np.arange(RPC)
    h0 = half0_nodes
    h1 = all_nodes[~np.isin(all_nodes, h0)]
    for nodes, blocks in ((h0, list(range(S0_BLOCKS))),
                          (h1, list(range(S0_BLOCKS, BPC)))):
        assign = _pack_greedy(nodes, [cntA, cntB, cntC, cntD], blocks, cap)
        # blocks list indexes into _pack_greedy's local 0..nb-1 space
        posd = {}
        slot = {j: 0 for j in range(len(blocks))}
        for i in sorted(assign):
            j = assign[i]
            posd[i] = blocks[j] * BLK + slot[j]
            slot[j] += 1
        for i in nodes:
            pos[i] = posd[i]
    return pos


def _build_stream(e_pos, e_idx16, e_dd, K, bias_idx=None):
    """Returns (idx_wrapped [128, SL/16] i16, s2 [128, nch*128] bf16).
    e_dd: per-edge weight folded into the one-hot matrix (dinv[dst]).
    bias_idx: if set, one extra slot per block gathers this row and adds it
    (weight 1.0) to every dst position of the block (bias fold-in)."""
    nch = BPC * K
    SL = nch * BLK
    blk = e_pos // BLK
    o = np.argsort(blk, kind="stable")
    blk_s = blk[o]
    e_pos = e_pos[o]
    e_idx16 = e_idx16[o]
    dd = e_dd[o] if e_dd is not None else np.ones(len(o), np.float32)
    counts = np.bincount(blk_s, minlength=BPC)
    cap = K * BLK - (1 if bias_idx is not None else 0)
    assert counts.max() <= cap, (counts.max(), cap)
    starts = np.concatenate([[0], np.cumsum(counts)[:-1]])
    within = np.arange(len(blk_s)) - np.repeat(starts, counts)
    slot = blk_s * (K * BLK) + within

    idx_full = np.zeros(SL, dtype=np.int16)
    idx_full[slot] = e_idx16
    s2 = np.zeros((128, nch, 128), dtype=BF16)
    s2[slot % BLK, slot // BLK, (e_pos % BLK)] = dd.astype(BF16)
    if bias_idx is not None:
        for b in range(BPC):
            fs = b * (K * BLK) + counts[b]     # first free slot of block b
            idx_full[fs] = bias_idx
            s2[fs % BLK, fs // BLK, :] = np.ones(128, dtype=BF16)
    idx_w = np.tile(idx_full.reshape(SL // 16, 16).T, (8, 1)).copy()
    return idx_w, s2.reshape(128, nch * 128)


def _preprocess(x, edge_index, W1, b1, W2, b2):
    src = np.asarray(edge_index[0], dtype=np.int64)
    dst = np.asarray(edge_index[1], dtype=np.int64)
    loops = np.arange(N_NODES, dtype=np.int64)
    src_all = np.concatenate([src, loops])
    dst_all = np.concatenate([dst, loops])
    deg = np.bincount(dst_all, minlength=N_NODES).astype(np.float32)
    dinv = np.where(deg > 0, 1.0 / np.sqrt(deg), 0.0).astype(np.float32)

    core_of = dst_all // RPC

    perms = []
    core_edges = []
    cnts_ab = []
    for c in range(NCORES):
        m = core_of == c
        s_c = src_all[m]
        d_loc = (dst_all[m] - c * RPC).astype(np.int64)
        cntA = np.bincount(d_loc[s_c < SPLIT1], minlength=RPC)
        cntB = np.bincount(d_loc[s_c >= SPLIT1], minlength=RPC)
        perms.append(_pack_blocks(cntA, cntB))
        core_edges.append((s_c, d_loc))
        cnts_ab.append((cntA, cntB))

    permpos_global = np.empty(N_NODES, dtype=np.int64)
    for c in range(NCORES):
        permpos_global[c * RPC:(c + 1) * RPC] = perms[c]

    # pass 2: rebalance within halves, also evening C/D (src-half) counts
    half_global = permpos_global < S0_ROWS
    perms2 = []
    for c in range(NCORES):
        s_c, d_loc = core_edges[c]
        cntA, cntB = cnts_ab[c]
        hsrc = half_global[s_c]
        cntC = np.bincount(d_loc[hsrc], minlength=RPC)
        cntD = np.bincount(d_loc[~hsrc], minlength=RPC)
        half0_nodes = np.where(perms[c] < S0_ROWS)[0]
        perms2.append(_pack_blocks4(cntA, cntB, cntC, cntD, half0_nodes))
    perms = perms2
    for c in range(NCORES):
        permpos_global[c * RPC:(c + 1) * RPC] = perms[c]

    def seg_K(e_pos, extra=0):
        counts = np.bincount(e_pos // BLK, minlength=BPC)
        return int(np.ceil((counts.max() + extra) / BLK))

    K1A = K1B = K2C = K2D = 1
    meta = []
    for c in range(NCORES):
        s_c, d_loc = core_edges[c]
        pos_d = perms[c][d_loc]
        mA = s_c < SPLIT1
        src_r = s_c // RPC
        src_pos = permpos_global[s_c]   # core-local position (0..RPAD-1)
        mC = src_pos < S0_ROWS
        K1A = max(K1A, seg_K(pos_d[mA], 1))
        K1B = max(K1B, seg_K(pos_d[~mA]))
        K2C = max(K2C, seg_K(pos_d[mC], 1))
        K2D = max(K2D, seg_K(pos_d[~mC]))
        meta.append((s_c, d_loc, pos_d, mA, mC, src_r, src_pos))

    in_maps = []
    xs = (np.asarray(x, np.float32) * dinv[:, None])
    xT = np.zeros((D_IN, NPAD), dtype=BF16)
    xT[:, :N_NODES] = xs.T.astype(BF16)
    w1b = np.asarray(W1, np.float32).astype(BF16)
    w2b = np.asarray(W2, np.float32).astype(BF16)
    ident = np.eye(128, dtype=np.float32).astype(BF16)
    # bias rows: b1 as a gatherable z1-table row, b2 as a z2-table row
    b1row = np.zeros((128, D_HID), dtype=BF16)
    b1row[0, :] = np.asarray(b1, np.float32).astype(BF16)
    b2row = np.zeros((16, 128), dtype=BF16)
    b2row[0, :D_OUT] = np.asarray(b2, np.float32).astype(BF16)

    real = padded = 0
    for c in range(NCORES):
        s_c, d_loc, pos_d, mA, mC, src_r, src_pos = meta[c]
        dd = dinv[d_loc + c * RPC]    # dinv[dst] per edge
        i1a, s2a = _build_stream(pos_d[mA], s_c[mA].astype(np.int16),
                                 dd[mA], K1A, bias_idx=SPLIT1)
        i1b, s2b = _build_stream(pos_d[~mA],
                                 (s_c[~mA] - SPLIT1).astype(np.int16),
                                 dd[~mA], K1B)
        idxC = (src_r * CPR + src_pos).astype(np.int16)
        idxD = (src_r * CPR + (src_pos - S0_ROWS)).astype(np.int16)
        i2c, s2c = _build_stream(pos_d[mC], idxC[mC], dd[mC], K2C,
                                 bias_idx=S0_ROWS)
        i2d, s2d = _build_stream(pos_d[~mC], idxD[~mC], dd[~mC], K2D)

        dinvb = np.zeros((BLK, BPC), dtype=np.float32)
        nodes_at = np.full(RPAD, -1, dtype=np.int64)
        nodes_at[perms[c]] = np.arange(RPC)
        valid = nodes_at >= 0
        dv = np.zeros(RPAD, np.float32)
        dv[valid] = dinv[nodes_at[valid] + c * RPC]
        dinvb[:, :] = dv.reshape(BPC, BLK).T

        in_maps.append({
            "xT": xT, "w1": w1b, "w2": w2b, "ident": ident,
            "b1row": b1row, "b2row": b2row,
            "dinvb": dinvb,
            "i1a": i1a, "s2a": s2a, "i1b": i1b, "s2b": s2b,
            "i2c": i2c, "s2c": s2c, "i2d": i2d, "s2d": s2d,
        })
        real += len(s_c)
        padded += BLK * BPC * (K1A + K1B)

    LAST["K"] = (K1A, K1B, K2C, K2D)
    LAST["pad_frac"] = padded / real - 1.0
    return in_maps, perms, (K1A, K1B, K2C, K2D)


# ---------------- device program ----------------

def _build_program(K1A, K1B, K2C, K2D):
    dt = mybir.dt
    phases = int(os.environ.get("GCN_PHASES", "3"))
    nc = bacc.Bacc(None, target_bir_lowering=False, debug=False,
                   num_devices=NCORES, num_swdge_queues=NQ,
                   dynamic_dma_scratch_size=int(
                       os.environ.get("GCN_SCRATCH", "16384")))

    xT = nc.dram_tensor("xT", [D_IN, NPAD], dt.bfloat16, kind="ExternalInput")
    w1 = nc.dram_tensor("w1", [D_IN, D_HID], dt.bfloat16, kind="ExternalInput")
    w2 = nc.dram_tensor("w2", [D_HID, D_OUT], dt.bfloat16, kind="ExternalInput")
    ident = nc.dram_tensor("ident", [128, 128], dt.bfloat16, kind="ExternalInput")
    b1row = nc.dram_tensor("b1row", [128, D_HID], dt.bfloat16,
                           kind="ExternalInput")
    b2row = nc.dram_tensor("b2row", [16, 128], dt.bfloat16,
                           kind="ExternalInput")
    dinvb = nc.dram_tensor("dinvb", [128, BPC], dt.float32, kind="ExternalInput")

    def idx_t(name, K):
        return nc.dram_tensor(name, [128, BPC * K * BLK // 16], dt.int16,
                              kind="ExternalInput")

    def s2_t(name, K):
        return nc.dram_tensor(name, [128, BPC * K * BLK], dt.bfloat16,
                              kind="ExternalInput")

    i1a, s2a = idx_t("i1a", K1A), s2_t("s2a", K1A)
    i1b, s2b = idx_t("i1b", K1B), s2_t("s2b", K1B)
    i2c, s2c = idx_t("i2c", K2C), s2_t("s2c", K2C)
    i2d, s2d = idx_t("i2d", K2D), s2_t("s2d", K2D)

    out = nc.dram_tensor("out", [RPAD, D_OUT], dt.float32, kind="ExternalOutput")

    z1A = nc.dram_tensor("z1A", [SPLIT1 + 128, D_HID], dt.bfloat16)
    z1B = nc.dram_tensor("z1B", [NPAD - SPLIT1, D_HID], dt.bfloat16)
    z2in0 = nc.dram_tensor("z2in0", [CPR, 128], dt.bfloat16)
    z2in1 = nc.dram_tensor("z2in1", [CPR, 128], dt.bfloat16)
    z2P0 = nc.dram_tensor("z2P0", [Z2ROWS, 128], dt.bfloat16,
                          addr_space="Shared")
    z2P1 = nc.dram_tensor("z2P1", [Z2ROWS, 128], dt.bfloat16,
                          addr_space="Shared")

    qctr = [0]

    def next_q():
        q = qctr[0] % NQ
        qctr[0] += 1
        return q

    with tile.TileContext(nc) as tc:
        with tc.tile_pool(name="consts", bufs=1) as cp, \
             tc.tile_pool(name="ph0x", bufs=2) as xp, \
             tc.tile_pool(name="ph0o", bufs=2) as op0, \
             tc.tile_pool(name="gp", bufs=2) as gp, \
             tc.tile_pool(name="ixp", bufs=2) as ixp, \
             tc.tile_pool(name="csp", bufs=BPC + 1) as csp, \
             tc.tile_pool(name="s2p", bufs=2) as s2p, \
             tc.tile_pool(name="hp", bufs=2) as hp, \
             tc.tile_pool(name="zp", bufs=3) as zp, \
             tc.tile_pool(name="smp", bufs=4) as smp, \
             tc.tile_pool(name="psAcc", bufs=3, space="PSUM") as psAcc, \
             tc.tile_pool(name="psMisc", bufs=1, space="PSUM") as psMisc, \
             tc.tile_pool(name="psO", bufs=3, space="PSUM") as psO:
            w1t = cp.tile([128, 4, D_HID], dt.bfloat16)
            nc.sync.dma_start(
                w1t[:], w1.ap().rearrange("(k p) n -> p k n", p=128))
            w2t = cp.tile([128, 2, D_OUT], dt.bfloat16)
            nc.sync.dma_start(
                w2t[:], w2.ap().rearrange("(k p) n -> p k n", p=128))
            idt = cp.tile([128, 128], dt.bfloat16)
            nc.sync.dma_start(idt[:], ident[:, :])
            dvt = cp.tile([128, BPC], dt.float32)
            nc.sync.dma_start(dvt[:], dinvb[:, :])
            # bias rows into the gather tables / AG contributions
            nc.sync.dma_start(z1A.ap()[SPLIT1:SPLIT1 + 128, :], b1row.ap()[:, :])
            nc.sync.dma_start(z2in0.ap()[S0_ROWS:CPR, :], b2row.ap()[:, :])
            nc.sync.dma_start(z2in1.ap()[S0_ROWS:CPR, :], b2row.ap()[:, :])

            # ---------------- phases 1+2 stream tables ----------------
            seg1 = {
                "A": (K1A, i1a, s2a, z1A.ap()[:, :]),
                "B": (K1B, i1b, s2b, z1B.ap()[:, :]),
            }
            seg2 = {
                "C": (K2C, i2c, s2c, z2P0.ap()[:, :]),
                "D": (K2D, i2d, s2d, z2P1.ap()[:, :]),
            }
            gtiles = {}
            s2tiles = {}

            def ensure_g(layer, s, pi):
                key = (layer, s, pi)
                if key in gtiles:
                    return gtiles[key]
                K, idrm, s2drm, zview = (seg1 if layer == 1 else seg2)[s]
                felem = D_HID if layer == 1 else 128
                SL = BPC * K * BLK
                n = min(PIECE, SL - pi * PIECE)
                off = pi * (PIECE // 16)
                it = ixp.tile([128, PIECE // 16], dt.int16,
                              tag=f"i{layer}{s}")
                nc.scalar.dma_start(
                    it[:, :n // 16], idrm.ap()[:, off:off + n // 16])
                gt = gp.tile([128, PIECE // 128, felem], dt.bfloat16,
                             tag=f"g{layer}{s}")
                nc.gpsimd.dma_gather(
                    gt[:, :n // 128, :], zview, it[:, :n // 16],
                    n, n, felem, queue_num=next_q())
                gtiles[key] = gt
                return gt

            def ensure_s2(layer, s, pi):
                key = (layer, s, pi)
                if key in s2tiles:
                    return s2tiles[key]
                K, idrm, s2drm, zview = (seg1 if layer == 1 else seg2)[s]
                nch = BPC * K
                n = min(S2CH, nch - pi * S2CH)
                st = s2p.tile([128, S2CH * 128], dt.bfloat16, tag=f"s{layer}{s}")
                nc.scalar.dma_start(
                    st[:, :n * 128],
                    s2drm.ap()[:, pi * S2CH * 128:(pi * S2CH + n) * 128])
                s2tiles[key] = st
                return st

            def l1_block(b):
                # psum accumulates sum_e dinv[d]*dinv[s]*z1[s] + b1 directly
                hps = psAcc.tile([128, D_HID], dt.float32, tag="acc")
                for s in ("A", "B"):
                    K = seg1[s][0]
                    for k in range(K):
                        ci = b * K + k
                        gpi, gpos = divmod(ci * BLK, PIECE)
                        spi, spos = divmod(ci, S2CH)
                        gt = ensure_g(1, s, gpi)
                        st = ensure_s2(1, s, spi)
                        nc.tensor.matmul(
                            hps[:],
                            st[:, spos * 128:(spos + 1) * 128],
                            gt[:, (gpos // BLK), :],
                            start=(s == "A" and k == 0),
                            stop=(s == "B" and k == K1B - 1))
                hr = hp.tile([128, D_HID], dt.bfloat16, tag="hr")
                nc.scalar.activation(
                    hr[:], hps[:], mybir.ActivationFunctionType.Relu)
                hT = hp.tile([128, 2, 128], dt.bfloat16, tag="hT")
                for h in range(2):
                    tps = psMisc.tile([128, 128], dt.bfloat16, tag="tps")
                    nc.tensor.transpose(
                        tps[:], hr[:, h * 128:(h + 1) * 128], idt[:])
                    nc.scalar.copy(hT[:, h, :], tps[:])
                zps = psMisc.tile([128, D_OUT], dt.float32, tag="zps")
                for h in range(2):
                    nc.tensor.matmul(
                        zps[:], hT[:, h, :], w2t[:, h, :],
                        start=(h == 0), stop=(h == 1))
                z2s = zp.tile([128, 128], dt.bfloat16, tag="z2s")
                nc.scalar.activation(
                    z2s[:, :D_OUT], zps[:],
                    mybir.ActivationFunctionType.Copy, scale=dvt[:, b:b + 1])
                if b < S0_BLOCKS:
                    nc.sync.dma_start(
                        z2in0.ap()[b * BLK:(b + 1) * BLK, :], z2s[:])
                else:
                    bb = b - S0_BLOCKS
                    nc.sync.dma_start(
                        z2in1.ap()[bb * BLK:(bb + 1) * BLK, :], z2s[:])

            cstash = {}

            def l2cd_block(b, s):
                ops = psO.tile([128, D_OUT], dt.float32, tag="ops")
                K = seg2[s][0]
                for k in range(K):
                    ci = b * K + k
                    gpi, gpos = divmod(ci * BLK, PIECE)
                    spi, spos = divmod(ci, S2CH)
                    gt = ensure_g(2, s, gpi)
                    st = ensure_s2(2, s, spi)
                    nc.tensor.matmul(
                        ops[:],
                        st[:, spos * 128:(spos + 1) * 128],
                        gt[:, (gpos // BLK), :D_OUT],
                        start=(k == 0), stop=(k == K - 1))
                return ops

            def l2c_block(b):
                ops = l2cd_block(b, "C")
                cs = csp.tile([128, D_OUT], dt.float32, tag="cs")
                nc.scalar.copy(cs[:], ops[:])
                cstash[b] = cs

            def l2d_block(b):
                ops = l2cd_block(b, "D")
                t2 = smp.tile([128, D_OUT], dt.float32, tag="t2")
                nc.vector.tensor_tensor(
                    t2[:], ops[:], cstash[b][:], op=mybir.AluOpType.add)
                nm = smp.tile([128, 1], dt.float32, tag="nm")
                nc.vector.reduce_max(
                    nm[:], t2[:], axis=mybir.AxisListType.X, negate=True)
                ex = smp.tile([128, D_OUT], dt.float32, tag="ex")
                sm = smp.tile([128, 1], dt.float32, tag="sm")
                nc.scalar.activation(
                    ex[:], t2[:], mybir.ActivationFunctionType.Exp,
                    bias=nm[:], accum_out=sm[:])
                rc = smp.tile([128, 1], dt.float32, tag="rc")
                nc.vector.reciprocal(rc[:], sm[:])
                ot = smp.tile([128, D_OUT], dt.float32, tag="ot")
                nc.vector.tensor_scalar(
                    ot[:], ex[:], rc[:], None, op0=mybir.AluOpType.mult)
                nc.sync.dma_start(out.ap()[b * BLK:(b + 1) * BLK, :], ot[:])

            # ---------------- phase 0: z1 = xT^T @ W1 (A half then B half) ---
            z1Av = z1A.ap()[0:SPLIT1, :].rearrange("(n p) f -> p n f", p=128)
            z1Bv = z1B.ap().rearrange("(n p) f -> p n f", p=128)
            NB_A = SPLIT1 // BLK
            GB = 7
            GRP = 8

            def phase0_range(glo, ghi):
                for g0 in range(glo, ghi, GRP):
                    gb = min(GRP, NBLOCKS - g0)
                    if gb <= 0:
                        break
                    xg = xp.tile([128, 4, GRP * BLK], dt.bfloat16, tag="xg")
                    nc.sync.dma_start(
                        xg[:, :, :gb * BLK],
                        xT.ap().rearrange("(k p) n -> p k n", p=128)
                        [:, :, g0 * BLK:(g0 + gb) * BLK])
                    for b0 in range(0, gb, GB):
                        nb = min(GB, gb - b0)
                        zo = op0.tile([128, GB, D_HID], dt.bfloat16, tag="zo")
                        for i in range(nb):
                            ps = psAcc.tile([128, D_HID], dt.float32, tag="acc")
                            col = (b0 + i) * BLK
                            for k in range(4):
                                nc.tensor.matmul(
                                    ps[:],
                                    xg[:, k, col:col + BLK],
                                    w1t[:, k, :],
                                    start=(k == 0), stop=(k == 3))
                            nc.vector.tensor_copy(zo[:, i, :], ps[:])
                        lo, hi = g0 + b0, g0 + b0 + nb
                        if hi <= NB_A:
                            nc.sync.dma_start(z1Av[:, lo:hi, :], zo[:, :nb, :])
                        elif lo >= NB_A:
                            nc.sync.dma_start(
                                z1Bv[:, lo - NB_A:hi - NB_A, :], zo[:, :nb, :])
                        else:
                            na = NB_A - lo
                            nc.sync.dma_start(z1Av[:, lo:NB_A, :], zo[:, :na, :])
                            nc.sync.dma_start(
                                z1Bv[:, 0:hi - NB_A, :], zo[:, na:nb, :])

            pref = int(os.environ.get("GCN_PREF", "1"))
            nl1 = int(os.environ.get("GCN_L1BLOCKS", str(BPC)))
            if phases >= 1:
                # phase 0 A-half (z1A rows), then prefetch the first L1-A
                # gather pieces so SWDGE ramps while phase 0 B computes.
                phase0_range(0, 200)
                if pref:
                    ensure_s2(1, "A", 0)
                    ensure_g(1, "A", 0)
                    ensure_g(1, "A", 1)
                phase0_range(200, NBLOCKS)

                for b in range(min(S0_BLOCKS, nl1)):
                    l1_block(b)
                if phases >= 2:
                    nc.gpsimd.collective_compute(
                        "AllGather", mybir.AluOpType.bypass,
                        replica_groups=[list(range(NCORES))],
                        ins=[z2in0.ap().opt()],
                        outs=[z2P0.ap().opt()])
                ci = 0
                for b in range(S0_BLOCKS, min(BPC, nl1)):
                    l1_block(b)
                    if phases >= 3 and b >= S0_BLOCKS + 12 and ci < BPC:
                        l2c_block(ci)
                        ci += 1
                if phases >= 2:
                    nc.gpsimd.collective_compute(
                        "AllGather", mybir.AluOpType.bypass,
                        replica_groups=[list(range(NCORES))],
                        ins=[z2in1.ap().opt()],
                        outs=[z2P1.ap().opt()])
                if phases >= 3:
                    while ci < BPC:
                        l2c_block(ci)
                        ci += 1
                    for b in range(BPC):
                        l2d_block(b)

    nc.compile()
    return nc


# ---------------- entry point ----------------

def kernel(x, edge_index, W1, b1, W2, b2):
    x = np.asarray(x)
    edge_index = np.asarray(edge_index)
    in_maps, perms, Ks = _preprocess(x, edge_index, W1, b1, W2, b2)
    nc = _build_program(*Ks)

    trace = os.environ.get("GCN_TRACE", "0") == "1"
    if trace:
        trace = _install_trace_hook()
    res = run_bass_kernel_spmd(
        nc, in_maps, core_ids=list(range(NCORES)), trace=trace)
    LAST["exec_time_ns"] = res.exec_time_ns
    LAST["results"] = res

    out = np.empty((N_NODES, D_OUT), dtype=np.float32)
    for c in range(NCORES):
        oc = np.asarray(res.results[c]["out"], dtype=np.float32)
        out[c * RPC:(c + 1) * RPC] = oc[perms[c]]
    return out


# revision 21
# speedup vs baseline: 1.6993x; 1.6993x over previous
"""GCN (2-layer, PyG GCNConv semantics) on 8 Trainium2 NeuronCores.

Strategy (dst-sharded message passing):
  out = softmax( A @ relu(A @ (x W1) + b1) @ W2 + b2 ),  A = D^-1/2 (Adj+I) D^-1/2

  - Host: degrees/dinv, self-loops appended as ordinary edges, edges
    partitioned by destination core (6250 dst rows per core), each core's
    dst nodes permuted into 50 load-balanced blocks of 128.  Per-edge
    gather indices (int16) and one-hot segment-sum matrices (bf16, with
    dinv[dst] folded in; bias rows folded in as extra "edges") are
    precomputed on the host and streamed to the device.
  - Phase 0 (on-device, redundant per core): z1 = (dinv*x) @ W1 in bf16,
    stored to local HBM (the layer-1 gather table).
  - Phase 1: per-edge dma_gather of z1 rows (4 SWDGE queues round-robin,
    4096-row pieces); segment-sum via TensorE matmuls h += S^T @ G
    (S = one-hot with dinv[dst]); relu on ScalarE; z2 = dinv * (h @ W2).
  - AllGather of z2 (bf16, rows padded to 128 cols) across the 8 cores
    in two row-slices.
  - Phase 2: per-edge dma_gather of z2 rows (bf16 256B rows), segment-sum
    to output blocks, softmax, DMA out.

kernel(**inputs) -> np.ndarray is self-contained (shapes hardcoded).
"""

import os
import sys
import types

sys.path.insert(0, "/opt/trn_rl_repo")

import numpy as np
import ml_dtypes

from concourse import bass, mybir, bacc, tile
from concourse.bass_utils import run_bass_kernel_spmd

BF16 = ml_dtypes.bfloat16

# ---------------- problem constants (hardcoded) ----------------
N_NODES = 50000
D_IN, D_HID, D_OUT = 512, 256, 64
NCORES = 8
RPC = N_NODES // NCORES          # 6250 dst rows per core
BLK = 128
BPC = 50                         # blocks per core (spare slots for balancing)
RPAD = BPC * BLK                 # 6400
NPAD = ((N_NODES + BLK - 1) // BLK) * BLK   # 50048 (391 node blocks)
NBLOCKS = NPAD // BLK            # 391
SPLIT1 = 24960                   # L1 gather src split (block-aligned, int16-safe)
S0_ROWS = 3200                   # AG slice 0: perm positions [0, 3200) = 25 blocks
S1_ROWS = RPAD - S0_ROWS         # 3200: positions [3200, 6400) = 25 blocks
S0_BLOCKS = S0_ROWS // BLK       # 25
CPR = S0_ROWS + 16               # rows per AG contribution (16 = b2 bias pad)
Z2ROWS = NCORES * CPR            # 25728 rows per z2 table
PIECE = int(os.environ.get("GCN_PIECE", "1024"))   # slots per dma_gather
                                 # (>1024 overflows the SWDGE ring: hangs)
S2CH = 16                        # one-hot chunks per S2 stream DMA piece
NQ = 4                           # SWDGE queues (ucode max)

LAST = {}                        # test harness introspection


def _install_trace_hook():
    try:
        mod = types.ModuleType("antenv.axon_hooks")
        hook = [None]
        mod.set_axon_ntff_profile_hook = lambda h: hook.__setitem__(0, h)
        mod.get_axon_ntff_profile_hook = lambda: hook[0]
        sys.modules["antenv.axon_hooks"] = mod
        import antenv
        antenv.axon_hooks = mod
        from trn_agent_boot.trn_boot import _ntff_profile_via_ctypes
        mod.set_axon_ntff_profile_hook(
            _ntff_profile_via_ctypes("/opt/axon/libaxon_pjrt.so"))
        return True
    except Exception:
        return False


# ---------------- host-side preprocessing ----------------

def _pack_greedy(node_ids, cnts, block_ids, cap):
    """Greedy k-dim balanced packing of node_ids into block_ids (<=128 each).
    cnts: [ndim, RPC] per-node counts. Returns {node: block}."""
    nd = len(cnts)
    nb = len(block_ids)
    tot = sum(c[node_ids] for c in cnts)
    order = node_ids[np.argsort(-tot, kind="stable")]
    sums = np.zeros((nd, nb), dtype=np.float64)
    cnt = np.zeros(nb, dtype=np.int64)
    assign = {}
    big = 1e18
    for i in order:
        score = np.max([(sums[d] + cnts[d][i]) / cap for d in range(nd)], axis=0)
        score = score + (sums.sum(axis=0) + tot[0] * 0) * 1e-7
        score = np.where(cnt < BLK, score, big)
        j = int(np.argmin(score))
        assign[i] = j
        cnt[j] += 1
        for d in range(nd):
            sums[d, j] += cnts[d][i]
    # repair per dim
    members = {j: [i for i, jj in assign.items() if jj == j] for j in range(nb)}
    for d in range(nd):
        for _ in range(2000):
            j = int(np.argmax(sums[d]))
            if sums[d, j] <= cap:
                break
            ms = members[j]
            pos_m = [i for i in ms if cnts[d][i] > 0]
            if not pos_m:
                break
            mv = min(pos_m, key=lambda i: cnts[d][i])
            tgt = np.where(cnt < BLK, sums[d], big)
            tgt[j] = big
            jt = int(np.argmin(tgt))
            if tgt[jt] >= big:
                break
            assign[mv] = jt
            members[j].remove(mv)
            members[jt].append(mv)
            cnt[j] -= 1
            cnt[jt] += 1
            for dd in range(nd):
                sums[dd, j] -= cnts[dd][mv]
                sums[dd, jt] += cnts[dd][mv]
    return assign


def _positions_from_assign(assign, block_ids):
    pos = {}
    slot = {j: 0 for j in block_ids}
    for i in sorted(assign):
        j = assign[i]
        pos[i] = j * BLK + slot[j]
        slot[j] += 1
    return pos


def _pack_blocks(cntA, cntB, cap=1148):
    nodes = np.arange(RPC)
    assign = _pack_greedy(nodes, [cntA, cntB], list(range(BPC)), cap)
    posd = _positions_from_assign(assign, list(range(BPC)))
    pos = np.empty(RPC, dtype=np.int64)
    for i in range(RPC):
        pos[i] = posd[i]
    return pos


def _pack_blocks4(cntA, cntB, cntC, cntD, half0_nodes, cap=1148):
    """Second pass: rebalance within halves on 4 dims."""
    pos = np.empty(RPC, dtype=np.int64)
    all_nodes = np.arange(RPC)
    h0 = half0_nodes
    h1 = all_nodes[~np.isin(all_nodes, h0)]
    for nodes, blocks in ((h0, list(range(S0_BLOCKS))),
                          (h1, list(range(S0_BLOCKS, BPC)))):
        assign = _pack_greedy(nodes, [cntA, cntB, cntC, cntD], blocks, cap)
        # blocks list indexes into _pack_greedy's local 0..nb-1 space
        posd = {}
        slot = {j: 0 for j in range(len(blocks))}
        for i in sorted(assign):
            j = assign[i]
            posd[i] = blocks[j] * BLK + slot[j]
            slot[j] += 1
        for i in nodes:
            pos[i] = posd[i]
    return pos


def _build_stream(e_pos, e_idx16, e_dd, K, bias_idx=None):
    """Returns (idx_wrapped [128, SL/16] i16, s2 [128, nch*128] bf16).
    e_dd: per-edge weight folded into the one-hot matrix (dinv[dst]).
    bias_idx: if set, one extra slot per block gathers this row and adds it
    (weight 1.0) to every dst position of the block (bias fold-in)."""
    nch = BPC * K
    SL = nch * BLK
    blk = e_pos // BLK
    o = np.argsort(blk, kind="stable")
    blk_s = blk[o]
    e_pos = e_pos[o]
    e_idx16 = e_idx16[o]
    dd = e_dd[o] if e_dd is not None else np.ones(len(o), np.float32)
    counts = np.bincount(blk_s, minlength=BPC)
    cap = K * BLK - (1 if bias_idx is not None else 0)
    assert counts.max() <= cap, (counts.max(), cap)
    starts = np.concatenate([[0], np.cumsum(counts)[:-1]])
    within = np.arange(len(blk_s)) - np.repeat(starts, counts)
    slot = blk_s * (K * BLK) + within

    idx_full = np.zeros(SL, dtype=np.int16)
    idx_full[slot] = e_idx16
    s2 = np.zeros((128, nch, 128), dtype=BF16)
    s2[slot % BLK, slot // BLK, (e_pos % BLK)] = dd.astype(BF16)
    if bias_idx is not None:
        for b in range(BPC):
            fs = b * (K * BLK) + counts[b]     # first free slot of block b
            idx_full[fs] = bias_idx
            s2[fs % BLK, fs // BLK, :] = np.ones(128, dtype=BF16)
    idx_w = np.tile(idx_full.reshape(SL // 16, 16).T, (8, 1)).copy()
    return idx_w, s2.reshape(128, nch * 128)


def _preprocess(x, edge_index, W1, b1, W2, b2):
    src = np.asarray(edge_index[0], dtype=np.int64)
    dst = np.asarray(edge_index[1], dtype=np.int64)
    loops = np.arange(N_NODES, dtype=np.int64)
    src_all = np.concatenate([src, loops])
    dst_all = np.concatenate([dst, loops])
    deg = np.bincount(dst_all, minlength=N_NODES).astype(np.float32)
    dinv = np.where(deg > 0, 1.0 / np.sqrt(deg), 0.0).astype(np.float32)

    core_of = dst_all // RPC

    perms = []
    core_edges = []
    cnts_ab = []
    for c in range(NCORES):
        m = core_of == c
        s_c = src_all[m]
        d_loc = (dst_all[m] - c * RPC).astype(np.int64)
        cntA = np.bincount(d_loc[s_c < SPLIT1], minlength=RPC)
        cntB = np.bincount(d_loc[s_c >= SPLIT1], minlength=RPC)
        perms.append(_pack_blocks(cntA, cntB))
        core_edges.append((s_c, d_loc))
        cnts_ab.append((cntA, cntB))

    permpos_global = np.empty(N_NODES, dtype=np.int64)
    for c in range(NCORES):
        permpos_global[c * RPC:(c + 1) * RPC] = perms[c]

    # pass 2: rebalance within halves, also evening C/D (src-half) counts
    half_global = permpos_global < S0_ROWS
    perms2 = []
    for c in range(NCORES):
        s_c, d_loc = core_edges[c]
        cntA, cntB = cnts_ab[c]
        hsrc = half_global[s_c]
        cntC = np.bincount(d_loc[hsrc], minlength=RPC)
        cntD = np.bincount(d_loc[~hsrc], minlength=RPC)
        half0_nodes = np.where(perms[c] < S0_ROWS)[0]
        perms2.append(_pack_blocks4(cntA, cntB, cntC, cntD, half0_nodes))
    perms = perms2
    for c in range(NCORES):
        permpos_global[c * RPC:(c + 1) * RPC] = perms[c]

    def seg_K(e_pos, extra=0):
        counts = np.bincount(e_pos // BLK, minlength=BPC)
        return int(np.ceil((counts.max() + extra) / BLK))

    K1A = K1B = K2C = K2D = 1
    meta = []
    for c in range(NCORES):
        s_c, d_loc = core_edges[c]
        pos_d = perms[c][d_loc]
        mA = s_c < SPLIT1
        src_r = s_c // RPC
        src_pos = permpos_global[s_c]   # core-local position (0..RPAD-1)
        mC = src_pos < S0_ROWS
        K1A = max(K1A, seg_K(pos_d[mA], 1))
        K1B = max(K1B, seg_K(pos_d[~mA]))
        K2C = max(K2C, seg_K(pos_d[mC], 1))
        K2D = max(K2D, seg_K(pos_d[~mC]))
        meta.append((s_c, d_loc, pos_d, mA, mC, src_r, src_pos))

    in_maps = []
    xs = (np.asarray(x, np.float32) * dinv[:, None])
    xT = np.zeros((D_IN, NPAD), dtype=BF16)
    xT[:, :N_NODES] = xs.T.astype(BF16)
    w1b = np.asarray(W1, np.float32).astype(BF16)
    w2b = np.asarray(W2, np.float32).astype(BF16)
    ident = np.eye(128, dtype=np.float32).astype(BF16)
    # bias rows: b1 as a gatherable z1-table row, b2 as a z2-table row
    b1row = np.zeros((128, D_HID), dtype=BF16)
    b1row[0, :] = np.asarray(b1, np.float32).astype(BF16)
    b2row = np.zeros((16, 128), dtype=BF16)
    b2row[0, :D_OUT] = np.asarray(b2, np.float32).astype(BF16)

    real = padded = 0
    for c in range(NCORES):
        s_c, d_loc, pos_d, mA, mC, src_r, src_pos = meta[c]
        dd = dinv[d_loc + c * RPC]    # dinv[dst] per edge
        i1a, s2a = _build_stream(pos_d[mA], s_c[mA].astype(np.int16),
                                 dd[mA], K1A, bias_idx=SPLIT1)
        i1b, s2b = _build_stream(pos_d[~mA],
                                 (s_c[~mA] - SPLIT1).astype(np.int16),
                                 dd[~mA], K1B)
        idxC = (src_r * CPR + src_pos).astype(np.int16)
        idxD = (src_r * CPR + (src_pos - S0_ROWS)).astype(np.int16)
        i2c, s2c = _build_stream(pos_d[mC], idxC[mC], dd[mC], K2C,
                                 bias_idx=S0_ROWS)
        i2d, s2d = _build_stream(pos_d[~mC], idxD[~mC], dd[~mC], K2D)

        dinvb = np.zeros((BLK, BPC), dtype=np.float32)
        nodes_at = np.full(RPAD, -1, dtype=np.int64)
        nodes_at[perms[c]] = np.arange(RPC)
        valid = nodes_at >= 0
        dv = np.zeros(RPAD, np.float32)
        dv[valid] = dinv[nodes_at[valid] + c * RPC]
        dinvb[:, :] = dv.reshape(BPC, BLK).T

        in_maps.append({
            "xT": xT, "w1": w1b, "w2": w2b, "ident": ident,
            "b1row": b1row, "b2row": b2row,
            "dinvb": dinvb,
            "i1a": i1a, "s2a": s2a, "i1b": i1b, "s2b": s2b,
            "i2c": i2c, "s2c": s2c, "i2d": i2d, "s2d": s2d,
        })
        real += len(s_c)
        padded += BLK * BPC * (K1A + K1B)

    LAST["K"] = (K1A, K1B, K2C, K2D)
    LAST["pad_frac"] = padded / real - 1.0
    return in_maps, perms, (K1A, K1B, K2C, K2D)


# ---------------- device program ----------------

def _build_program(K1A, K1B, K2C, K2D):
    dt = mybir.dt
    phases = int(os.environ.get("GCN_PHASES", "3"))
    nc = bacc.Bacc(None, target_bir_lowering=False, debug=False,
                   num_devices=NCORES, num_swdge_queues=NQ,
                   dynamic_dma_scratch_size=int(
                       os.environ.get("GCN_SCRATCH", "16384")))

    xT = nc.dram_tensor("xT", [D_IN, NPAD], dt.bfloat16, kind="ExternalInput")
    w1 = nc.dram_tensor("w1", [D_IN, D_HID], dt.bfloat16, kind="ExternalInput")
    w2 = nc.dram_tensor("w2", [D_HID, D_OUT], dt.bfloat16, kind="ExternalInput")
    ident = nc.dram_tensor("ident", [128, 128], dt.bfloat16, kind="ExternalInput")
    b1row = nc.dram_tensor("b1row", [128, D_HID], dt.bfloat16,
                           kind="ExternalInput")
    b2row = nc.dram_tensor("b2row", [16, 128], dt.bfloat16,
                           kind="ExternalInput")
    dinvb = nc.dram_tensor("dinvb", [128, BPC], dt.float32, kind="ExternalInput")

    def idx_t(name, K):
        return nc.dram_tensor(name, [128, BPC * K * BLK // 16], dt.int16,
                              kind="ExternalInput")

    def s2_t(name, K):
        return nc.dram_tensor(name, [128, BPC * K * BLK], dt.bfloat16,
                              kind="ExternalInput")

    i1a, s2a = idx_t("i1a", K1A), s2_t("s2a", K1A)
    i1b, s2b = idx_t("i1b", K1B), s2_t("s2b", K1B)
    i2c, s2c = idx_t("i2c", K2C), s2_t("s2c", K2C)
    i2d, s2d = idx_t("i2d", K2D), s2_t("s2d", K2D)

    out = nc.dram_tensor("out", [RPAD, D_OUT], dt.float32, kind="ExternalOutput")

    z1A = nc.dram_tensor("z1A", [SPLIT1 + 128, D_HID], dt.bfloat16)
    z1B = nc.dram_tensor("z1B", [NPAD - SPLIT1, D_HID], dt.bfloat16)
    z2in0 = nc.dram_tensor("z2in0", [CPR, 128], dt.bfloat16)
    z2in1 = nc.dram_tensor("z2in1", [CPR, 128], dt.bfloat16)
    z2P0 = nc.dram_tensor("z2P0", [Z2ROWS, 128], dt.bfloat16,
                          addr_space="Shared")
    z2P1 = nc.dram_tensor("z2P1", [Z2ROWS, 128], dt.bfloat16,
                          addr_space="Shared")

    qctr = [0]

    def next_q():
        q = qctr[0] % NQ
        qctr[0] += 1
        return q

    with tile.TileContext(nc) as tc:
        with tc.tile_pool(name="consts", bufs=1) as cp, \
             tc.tile_pool(name="ph0x", bufs=2) as xp, \
             tc.tile_pool(name="ph0o", bufs=3) as op0, \
             tc.tile_pool(name="gp", bufs=4) as gp, \
             tc.tile_pool(name="csp", bufs=BPC + 1) as csp, \
             tc.tile_pool(name="s2p", bufs=2) as s2p, \
             tc.tile_pool(name="hp", bufs=2) as hp, \
             tc.tile_pool(name="zp", bufs=3) as zp, \
             tc.tile_pool(name="smp", bufs=4) as smp, \
             tc.tile_pool(name="psAcc", bufs=3, space="PSUM") as psAcc, \
             tc.tile_pool(name="psMisc", bufs=1, space="PSUM") as psMisc, \
             tc.tile_pool(name="psO", bufs=3, space="PSUM") as psO:
            w1t = cp.tile([128, 4, D_HID], dt.bfloat16)
            nc.sync.dma_start(
                w1t[:], w1.ap().rearrange("(k p) n -> p k n", p=128))
            w2t = cp.tile([128, 2, D_OUT], dt.bfloat16)
            nc.sync.dma_start(
                w2t[:], w2.ap().rearrange("(k p) n -> p k n", p=128))
            idt = cp.tile([128, 128], dt.bfloat16)
            nc.sync.dma_start(idt[:], ident[:, :])
            dvt = cp.tile([128, BPC], dt.float32)
            nc.sync.dma_start(dvt[:], dinvb[:, :])
            # bias rows into the gather tables / AG contributions
            nc.sync.dma_start(z1A.ap()[SPLIT1:SPLIT1 + 128, :], b1row.ap()[:, :])
            nc.sync.dma_start(z2in0.ap()[S0_ROWS:CPR, :], b2row.ap()[:, :])
            nc.sync.dma_start(z2in1.ap()[S0_ROWS:CPR, :], b2row.ap()[:, :])
            it1a = cp.tile([128, BPC * K1A * BLK // 16], dt.int16)
            nc.scalar.dma_start(it1a[:], i1a[:, :])
            it1b = cp.tile([128, BPC * K1B * BLK // 16], dt.int16)
            nc.scalar.dma_start(it1b[:], i1b[:, :])
            it2c = cp.tile([128, BPC * K2C * BLK // 16], dt.int16)
            nc.scalar.dma_start(it2c[:], i2c[:, :])
            it2d = cp.tile([128, BPC * K2D * BLK // 16], dt.int16)
            nc.scalar.dma_start(it2d[:], i2d[:, :])
            itabs = {(1, "A"): it1a, (1, "B"): it1b,
                     (2, "C"): it2c, (2, "D"): it2d}

            # ---------------- phases 1+2 stream tables ----------------
            seg1 = {
                "A": (K1A, i1a, s2a, z1A.ap()[:, :]),
                "B": (K1B, i1b, s2b, z1B.ap()[:, :]),
            }
            seg2 = {
                "C": (K2C, i2c, s2c, z2P0.ap()[:, :]),
                "D": (K2D, i2d, s2d, z2P1.ap()[:, :]),
            }
            gtiles = {}
            s2tiles = {}

            def ensure_g(layer, s, pi):
                key = (layer, s, pi)
                if key in gtiles:
                    return gtiles[key]
                K, idrm, s2drm, zview = (seg1 if layer == 1 else seg2)[s]
                felem = D_HID if layer == 1 else 128
                SL = BPC * K * BLK
                n = min(PIECE, SL - pi * PIECE)
                off = pi * (PIECE // 16)
                it = itabs[(layer, s)]
                gt = gp.tile([128, PIECE // 128, felem], dt.bfloat16,
                             tag=f"g{layer}{s}")
                nc.gpsimd.dma_gather(
                    gt[:, :n // 128, :], zview, it[:, off:off + n // 16],
                    n, n, felem, queue_num=next_q())
                gtiles[key] = gt
                return gt

            def ensure_s2(layer, s, pi):
                key = (layer, s, pi)
                if key in s2tiles:
                    return s2tiles[key]
                K, idrm, s2drm, zview = (seg1 if layer == 1 else seg2)[s]
                nch = BPC * K
                n = min(S2CH, nch - pi * S2CH)
                st = s2p.tile([128, S2CH * 128], dt.bfloat16, tag=f"s{layer}{s}")
                nc.scalar.dma_start(
                    st[:, :n * 128],
                    s2drm.ap()[:, pi * S2CH * 128:(pi * S2CH + n) * 128])
                s2tiles[key] = st
                return st

            def l1_block(b):
                # psum accumulates sum_e dinv[d]*dinv[s]*z1[s] + b1 directly
                hps = psAcc.tile([128, D_HID], dt.float32, tag="acc")
                for s in ("A", "B"):
                    K = seg1[s][0]
                    for k in range(K):
                        ci = b * K + k
                        gpi, gpos = divmod(ci * BLK, PIECE)
                        spi, spos = divmod(ci, S2CH)
                        gt = ensure_g(1, s, gpi)
                        st = ensure_s2(1, s, spi)
                        nc.tensor.matmul(
                            hps[:],
                            st[:, spos * 128:(spos + 1) * 128],
                            gt[:, (gpos // BLK), :],
                            start=(s == "A" and k == 0),
                            stop=(s == "B" and k == K1B - 1))
                hr = hp.tile([128, D_HID], dt.bfloat16, tag="hr")
                nc.scalar.activation(
                    hr[:], hps[:], mybir.ActivationFunctionType.Relu)
                hT = hp.tile([128, 2, 128], dt.bfloat16, tag="hT")
                for h in range(2):
                    tps = psMisc.tile([128, 128], dt.bfloat16, tag="tps")
                    nc.tensor.transpose(
                        tps[:], hr[:, h * 128:(h + 1) * 128], idt[:])
                    nc.scalar.copy(hT[:, h, :], tps[:])
                zps = psMisc.tile([128, D_OUT], dt.float32, tag="zps")
                for h in range(2):
                    nc.tensor.matmul(
                        zps[:], hT[:, h, :], w2t[:, h, :],
                        start=(h == 0), stop=(h == 1))
                z2s = zp.tile([128, 128], dt.bfloat16, tag="z2s")
                nc.scalar.activation(
                    z2s[:, :D_OUT], zps[:],
                    mybir.ActivationFunctionType.Copy, scale=dvt[:, b:b + 1])
                if b < S0_BLOCKS:
                    nc.sync.dma_start(
                        z2in0.ap()[b * BLK:(b + 1) * BLK, :], z2s[:])
                else:
                    bb = b - S0_BLOCKS
                    nc.sync.dma_start(
                        z2in1.ap()[bb * BLK:(bb + 1) * BLK, :], z2s[:])

            cstash = {}

            def l2cd_block(b, s):
                ops = psO.tile([128, D_OUT], dt.float32, tag="ops")
                K = seg2[s][0]
                for k in range(K):
                    ci = b * K + k
                    gpi, gpos = divmod(ci * BLK, PIECE)
                    spi, spos = divmod(ci, S2CH)
                    gt = ensure_g(2, s, gpi)
                    st = ensure_s2(2, s, spi)
                    nc.tensor.matmul(
                        ops[:],
                        st[:, spos * 128:(spos + 1) * 128],
                        gt[:, (gpos // BLK), :D_OUT],
                        start=(k == 0), stop=(k == K - 1))
                return ops

            def l2c_block(b):
                ops = l2cd_block(b, "C")
                cs = csp.tile([128, D_OUT], dt.float32, tag="cs")
                nc.scalar.copy(cs[:], ops[:])
                cstash[b] = cs

            def l2d_block(b):
                ops = l2cd_block(b, "D")
                t2 = smp.tile([128, D_OUT], dt.float32, tag="t2")
                nc.vector.tensor_tensor(
                    t2[:], ops[:], cstash[b][:], op=mybir.AluOpType.add)
                nm = smp.tile([128, 1], dt.float32, tag="nm")
                nc.vector.reduce_max(
                    nm[:], t2[:], axis=mybir.AxisListType.X, negate=True)
                ex = smp.tile([128, D_OUT], dt.float32, tag="ex")
                sm = smp.tile([128, 1], dt.float32, tag="sm")
                nc.scalar.activation(
                    ex[:], t2[:], mybir.ActivationFunctionType.Exp,
                    bias=nm[:], accum_out=sm[:])
                rc = smp.tile([128, 1], dt.float32, tag="rc")
                nc.vector.reciprocal(rc[:], sm[:])
                ot = smp.tile([128, D_OUT], dt.float32, tag="ot")
                nc.vector.tensor_scalar(
                    ot[:], ex[:], rc[:], None, op0=mybir.AluOpType.mult)
                nc.sync.dma_start(out.ap()[b * BLK:(b + 1) * BLK, :], ot[:])

            # ---------------- phase 0: z1 = xT^T @ W1 (A half then B half) ---
            z1Av = z1A.ap()[0:SPLIT1, :].rearrange("(n p) f -> p n f", p=128)
            z1Bv = z1B.ap().rearrange("(n p) f -> p n f", p=128)
            NB_A = SPLIT1 // BLK
            GB = 7
            GRP = 16

            def phase0_range(glo, ghi):
                for g0 in range(glo, ghi, GRP):
                    gb = min(GRP, NBLOCKS - g0)
                    if gb <= 0:
                        break
                    xg = xp.tile([128, 4, GRP * BLK], dt.bfloat16, tag="xg")
                    nc.sync.dma_start(
                        xg[:, :, :gb * BLK],
                        xT.ap().rearrange("(k p) n -> p k n", p=128)
                        [:, :, g0 * BLK:(g0 + gb) * BLK])
                    for b0 in range(0, gb, GB):
                        nb = min(GB, gb - b0)
                        zo = op0.tile([128, GB, D_HID], dt.bfloat16, tag="zo")
                        for i in range(nb):
                            ps = psAcc.tile([128, D_HID], dt.float32, tag="acc")
                            col = (b0 + i) * BLK
                            for k in range(4):
                                nc.tensor.matmul(
                                    ps[:],
                                    xg[:, k, col:col + BLK],
                                    w1t[:, k, :],
                                    start=(k == 0), stop=(k == 3))
                            nc.vector.tensor_copy(zo[:, i, :], ps[:])
                        lo, hi = g0 + b0, g0 + b0 + nb
                        if hi <= NB_A:
                            nc.sync.dma_start(z1Av[:, lo:hi, :], zo[:, :nb, :])
                        elif lo >= NB_A:
                            nc.sync.dma_start(
                                z1Bv[:, lo - NB_A:hi - NB_A, :], zo[:, :nb, :])
                        else:
                            na = NB_A - lo
                            nc.sync.dma_start(z1Av[:, lo:NB_A, :], zo[:, :na, :])
                            nc.sync.dma_start(
                                z1Bv[:, 0:hi - NB_A, :], zo[:, na:nb, :])

            pref = int(os.environ.get("GCN_PREF", "1"))
            nl1 = int(os.environ.get("GCN_L1BLOCKS", str(BPC)))
            if phases >= 1:
                # phase 0 A-half (z1A rows), then prefetch the first L1-A
                # gather pieces so SWDGE ramps while phase 0 B computes.
                phase0_range(0, 200)
                if pref:
                    ensure_s2(1, "A", 0)
                    for pi in range(4):
                        ensure_g(1, "A", pi)
                phase0_range(208, NBLOCKS)

                for b in range(min(S0_BLOCKS, nl1)):
                    l1_block(b)
                if phases >= 2:
                    nc.gpsimd.collective_compute(
                        "AllGather", mybir.AluOpType.bypass,
                        replica_groups=[list(range(NCORES))],
                        ins=[z2in0.ap().opt()],
                        outs=[z2P0.ap().opt()])
                ci = 0
                for b in range(S0_BLOCKS, min(BPC, nl1)):
                    l1_block(b)
                    if phases >= 3 and b >= S0_BLOCKS + 12 and ci < BPC:
                        l2c_block(ci)
                        ci += 1
                if phases >= 2:
                    nc.gpsimd.collective_compute(
                        "AllGather", mybir.AluOpType.bypass,
                        replica_groups=[list(range(NCORES))],
                        ins=[z2in1.ap().opt()],
                        outs=[z2P1.ap().opt()])
                if phases >= 3:
                    while ci < BPC:
                        l2c_block(ci)
                        ci += 1
                    for b in range(BPC):
                        l2d_block(b)

    nc.compile()
    return nc


# ---------------- entry point ----------------

def kernel(x, edge_index, W1, b1, W2, b2):
    x = np.asarray(x)
    edge_index = np.asarray(edge_index)
    in_maps, perms, Ks = _preprocess(x, edge_index, W1, b1, W2, b2)
    nc = _build_program(*Ks)

    trace = os.environ.get("GCN_TRACE", "0") == "1"
    if trace:
        trace = _install_trace_hook()
    res = run_bass_kernel_spmd(
        nc, in_maps, core_ids=list(range(NCORES)), trace=trace)
    LAST["exec_time_ns"] = res.exec_time_ns
    LAST["results"] = res

    out = np.empty((N_NODES, D_OUT), dtype=np.float32)
    for c in range(NCORES):
        oc = np.asarray(res.results[c]["out"], dtype=np.float32)
        out[c * RPC:(c + 1) * RPC] = oc[perms[c]]
    return out


# revision 37
# speedup vs baseline: 1.9776x; 1.1638x over previous
"""GCN (2-layer, PyG GCNConv semantics) on 8 Trainium2 NeuronCores.

Strategy (dst-sharded message passing):
  out = softmax( A @ relu(A @ (x W1) + b1) @ W2 + b2 ),  A = D^-1/2 (Adj+I) D^-1/2

  - Host: degrees/dinv, self-loops appended as ordinary edges, edges
    partitioned by destination core (6250 dst rows per core), each core's
    dst nodes permuted into 50 load-balanced blocks of 128.  Per-edge
    gather indices (int16) and one-hot segment-sum matrices (bf16, with
    dinv[dst] folded in; bias rows folded in as extra "edges") are
    precomputed on the host and streamed to the device.
  - Phase 0 (on-device, redundant per core): z1 = (dinv*x) @ W1 in bf16,
    stored to local HBM (the layer-1 gather table).
  - Phase 1: per-edge dma_gather of z1 rows (4 SWDGE queues round-robin,
    4096-row pieces); segment-sum via TensorE matmuls h += S^T @ G
    (S = one-hot with dinv[dst]); relu on ScalarE; z2 = dinv * (h @ W2).
  - AllGather of z2 (bf16, rows padded to 128 cols) across the 8 cores
    in two row-slices.
  - Phase 2: per-edge dma_gather of z2 rows (bf16 256B rows), segment-sum
    to output blocks, softmax, DMA out.

kernel(**inputs) -> np.ndarray is self-contained (shapes hardcoded).
"""

import os
import sys
import types

sys.path.insert(0, "/opt/trn_rl_repo")

import numpy as np
import ml_dtypes

from concourse import bass, mybir, bacc, tile
from concourse.bass_utils import run_bass_kernel_spmd

BF16 = ml_dtypes.bfloat16
FP8 = ml_dtypes.float8_e4m3   # matches mybir dt.float8e4 (TRN FP8_EXP4)
USE_FP8 = os.environ.get("GCN_FP8", "1") == "1"      # z1 table + L1 one-hots
USE_FP8S2 = os.environ.get("GCN_FP8S2", "1") == "1"  # L2 one-hots (mixed mm)

# ---------------- problem constants (hardcoded) ----------------
N_NODES = 50000
D_IN, D_HID, D_OUT = 512, 256, 64
NCORES = 8
RPC = N_NODES // NCORES          # 6250 dst rows per core
BLK = 128
BPC = 50                         # blocks per core (spare slots for balancing)
RPAD = BPC * BLK                 # 6400
NPAD = ((N_NODES + BLK - 1) // BLK) * BLK   # 50048 (391 node blocks)
NBLOCKS = NPAD // BLK            # 391
SPLIT1 = 24960                   # L1 gather src split (block-aligned, int16-safe)
S0_ROWS = 3200                   # AG slice 0: perm positions [0, 3200) = 25 blocks
S1_ROWS = RPAD - S0_ROWS         # 3200: positions [3200, 6400) = 25 blocks
S0_BLOCKS = S0_ROWS // BLK       # 25
CPR = S0_ROWS + 16               # rows per AG contribution (16 = b2 bias pad)
Z2ROWS = NCORES * CPR            # 25728 rows per z2 table
PIECE = int(os.environ.get("GCN_PIECE", "1024"))   # slots per dma_gather
                                 # (>1024 overflows the SWDGE ring: hangs)
S2CH = 32                        # one-hot chunks per S2 stream DMA piece
NQ = 4                           # SWDGE queues (ucode max)

LAST = {}                        # test harness introspection


def _install_trace_hook():
    try:
        mod = types.ModuleType("antenv.axon_hooks")
        hook = [None]
        mod.set_axon_ntff_profile_hook = lambda h: hook.__setitem__(0, h)
        mod.get_axon_ntff_profile_hook = lambda: hook[0]
        sys.modules["antenv.axon_hooks"] = mod
        import antenv
        antenv.axon_hooks = mod
        from trn_agent_boot.trn_boot import _ntff_profile_via_ctypes
        mod.set_axon_ntff_profile_hook(
            _ntff_profile_via_ctypes("/opt/axon/libaxon_pjrt.so"))
        return True
    except Exception:
        return False


# ---------------- host-side preprocessing ----------------

def _pack_greedy(node_ids, cnts, block_ids, cap):
    """Greedy k-dim balanced packing of node_ids into block_ids (<=128 each).
    cnts: [ndim, RPC] per-node counts. Returns {node: block}."""
    nd = len(cnts)
    nb = len(block_ids)
    tot = sum(c[node_ids] for c in cnts)
    order = node_ids[np.argsort(-tot, kind="stable")]
    sums = np.zeros((nd, nb), dtype=np.float64)
    cnt = np.zeros(nb, dtype=np.int64)
    assign = {}
    big = 1e18
    for i in order:
        score = np.max([(sums[d] + cnts[d][i]) / cap for d in range(nd)], axis=0)
        score = score + (sums.sum(axis=0) + tot[0] * 0) * 1e-7
        score = np.where(cnt < BLK, score, big)
        j = int(np.argmin(score))
        assign[i] = j
        cnt[j] += 1
        for d in range(nd):
            sums[d, j] += cnts[d][i]
    # repair per dim
    members = {j: [i for i, jj in assign.items() if jj == j] for j in range(nb)}
    for d in range(nd):
        for _ in range(2000):
            j = int(np.argmax(sums[d]))
            if sums[d, j] <= cap:
                break
            ms = members[j]
            pos_m = [i for i in ms if cnts[d][i] > 0]
            if not pos_m:
                break
            mv = min(pos_m, key=lambda i: cnts[d][i])
            tgt = np.where(cnt < BLK, sums[d], big)
            tgt[j] = big
            jt = int(np.argmin(tgt))
            if tgt[jt] >= big:
                break
            assign[mv] = jt
            members[j].remove(mv)
            members[jt].append(mv)
            cnt[j] -= 1
            cnt[jt] += 1
            for dd in range(nd):
                sums[dd, j] -= cnts[dd][mv]
                sums[dd, jt] += cnts[dd][mv]
    return assign


def _positions_from_assign(assign, block_ids):
    pos = {}
    slot = {j: 0 for j in block_ids}
    for i in sorted(assign):
        j = assign[i]
        pos[i] = j * BLK + slot[j]
        slot[j] += 1
    return pos


def _pack_blocks(cntA, cntB, cap=1148):
    nodes = np.arange(RPC)
    assign = _pack_greedy(nodes, [cntA, cntB], list(range(BPC)), cap)
    posd = _positions_from_assign(assign, list(range(BPC)))
    pos = np.empty(RPC, dtype=np.int64)
    for i in range(RPC):
        pos[i] = posd[i]
    return pos


def _pack_blocks4(cntA, cntB, cntC, cntD, half0_nodes, cap=1148):
    """Second pass: rebalance within halves on 4 dims."""
    pos = np.empty(RPC, dtype=np.int64)
    all_nodes = np.arange(RPC)
    h0 = half0_nodes
    h1 = all_nodes[~np.isin(all_nodes, h0)]
    for nodes, blocks in ((h0, list(range(S0_BLOCKS))),
                          (h1, list(range(S0_BLOCKS, BPC)))):
        assign = _pack_greedy(nodes, [cntA, cntB, cntC, cntD], blocks, cap)
        # blocks list indexes into _pack_greedy's local 0..nb-1 space
        posd = {}
        slot = {j: 0 for j in range(len(blocks))}
        for i in sorted(assign):
            j = assign[i]
            posd[i] = blocks[j] * BLK + slot[j]
            slot[j] += 1
        for i in nodes:
            pos[i] = posd[i]
    return pos


def _build_stream(e_pos, e_idx16, e_dd, K, bias_idx=None, sdt=BF16):
    """Returns (idx_wrapped [128, SL/16] i16, s2 [128, nch*128] sdt).
    e_dd: per-edge weight folded into the one-hot matrix (dinv[dst]).
    bias_idx: if set, one extra slot per block gathers this row and adds it
    (weight 1.0) to every dst position of the block (bias fold-in)."""
    nch = BPC * K
    SL = nch * BLK
    blk = e_pos // BLK
    o = np.argsort(blk, kind="stable")
    blk_s = blk[o]
    e_pos = e_pos[o]
    e_idx16 = e_idx16[o]
    dd = e_dd[o] if e_dd is not None else np.ones(len(o), np.float32)
    counts = np.bincount(blk_s, minlength=BPC)
    cap = K * BLK - (1 if bias_idx is not None else 0)
    assert counts.max() <= cap, (counts.max(), cap)
    starts = np.concatenate([[0], np.cumsum(counts)[:-1]])
    within = np.arange(len(blk_s)) - np.repeat(starts, counts)
    slot = blk_s * (K * BLK) + within

    idx_full = np.zeros(SL, dtype=np.int16)
    idx_full[slot] = e_idx16
    s2 = np.zeros((128, nch, 128), dtype=sdt)
    s2[slot % BLK, slot // BLK, (e_pos % BLK)] = dd.astype(sdt)
    if bias_idx is not None:
        for b in range(BPC):
            fs = b * (K * BLK) + counts[b]     # first free slot of block b
            idx_full[fs] = bias_idx
            s2[fs % BLK, fs // BLK, :] = np.ones(128, dtype=sdt)
    idx_w = np.tile(idx_full.reshape(SL // 16, 16).T, (8, 1)).copy()
    return idx_w, s2.reshape(128, nch * 128)


def _preprocess(x, edge_index, W1, b1, W2, b2):
    src = np.asarray(edge_index[0], dtype=np.int64)
    dst = np.asarray(edge_index[1], dtype=np.int64)
    loops = np.arange(N_NODES, dtype=np.int64)
    src_all = np.concatenate([src, loops])
    dst_all = np.concatenate([dst, loops])
    deg = np.bincount(dst_all, minlength=N_NODES).astype(np.float32)
    dinv = np.where(deg > 0, 1.0 / np.sqrt(deg), 0.0).astype(np.float32)

    core_of = dst_all // RPC

    perms = []
    core_edges = []
    cnts_ab = []
    for c in range(NCORES):
        m = core_of == c
        s_c = src_all[m]
        d_loc = (dst_all[m] - c * RPC).astype(np.int64)
        cntA = np.bincount(d_loc[s_c < SPLIT1], minlength=RPC)
        cntB = np.bincount(d_loc[s_c >= SPLIT1], minlength=RPC)
        perms.append(_pack_blocks(cntA, cntB))
        core_edges.append((s_c, d_loc))
        cnts_ab.append((cntA, cntB))

    permpos_global = np.empty(N_NODES, dtype=np.int64)
    for c in range(NCORES):
        permpos_global[c * RPC:(c + 1) * RPC] = perms[c]

    # pass 2: rebalance within halves, also evening C/D (src-half) counts
    half_global = permpos_global < S0_ROWS
    perms2 = []
    for c in range(NCORES):
        s_c, d_loc = core_edges[c]
        cntA, cntB = cnts_ab[c]
        hsrc = half_global[s_c]
        cntC = np.bincount(d_loc[hsrc], minlength=RPC)
        cntD = np.bincount(d_loc[~hsrc], minlength=RPC)
        half0_nodes = np.where(perms[c] < S0_ROWS)[0]
        perms2.append(_pack_blocks4(cntA, cntB, cntC, cntD, half0_nodes))
    perms = perms2
    for c in range(NCORES):
        permpos_global[c * RPC:(c + 1) * RPC] = perms[c]

    def seg_K(e_pos, extra=0):
        counts = np.bincount(e_pos // BLK, minlength=BPC)
        return int(np.ceil((counts.max() + extra) / BLK))

    K1A = K1B = K2C = K2D = 1
    meta = []
    for c in range(NCORES):
        s_c, d_loc = core_edges[c]
        pos_d = perms[c][d_loc]
        mA = s_c < SPLIT1
        src_r = s_c // RPC
        src_pos = permpos_global[s_c]   # core-local position (0..RPAD-1)
        mC = src_pos < S0_ROWS
        K1A = max(K1A, seg_K(pos_d[mA], 1))
        K1B = max(K1B, seg_K(pos_d[~mA]))
        K2C = max(K2C, seg_K(pos_d[mC], 1))
        K2D = max(K2D, seg_K(pos_d[~mC]))
        meta.append((s_c, d_loc, pos_d, mA, mC, src_r, src_pos))

    in_maps = []
    xs = (np.asarray(x, np.float32) * dinv[:, None])
    xT = np.zeros((D_IN, NPAD), dtype=BF16)
    xT[:, :N_NODES] = xs.T.astype(BF16)
    w1b = np.asarray(W1, np.float32).astype(BF16)
    w2b = np.asarray(W2, np.float32).astype(BF16)
    ident = np.eye(128, dtype=np.float32).astype(BF16)
    # bias rows: b1 as a gatherable z1-table row, b2 as a z2-table row
    b1row = np.zeros((128, D_HID), dtype=FP8 if USE_FP8 else BF16)
    b1row[0, :] = np.asarray(b1, np.float32).astype(b1row.dtype)
    b2row = np.zeros((16, 128), dtype=BF16)
    b2row[0, :D_OUT] = np.asarray(b2, np.float32).astype(BF16)

    real = padded = 0
    for c in range(NCORES):
        s_c, d_loc, pos_d, mA, mC, src_r, src_pos = meta[c]
        dd = dinv[d_loc + c * RPC]    # dinv[dst] per edge
        s1dt = FP8 if USE_FP8 else BF16
        s2dt = FP8 if USE_FP8S2 else BF16
        i1a, s2a = _build_stream(pos_d[mA], s_c[mA].astype(np.int16),
                                 dd[mA], K1A, bias_idx=SPLIT1, sdt=s1dt)
        i1b, s2b = _build_stream(pos_d[~mA],
                                 (s_c[~mA] - SPLIT1).astype(np.int16),
                                 dd[~mA], K1B, sdt=s1dt)
        idxC = (src_r * CPR + src_pos).astype(np.int16)
        idxD = (src_r * CPR + (src_pos - S0_ROWS)).astype(np.int16)
        i2c, s2c = _build_stream(pos_d[mC], idxC[mC], dd[mC], K2C,
                                 bias_idx=S0_ROWS, sdt=s2dt)
        i2d, s2d = _build_stream(pos_d[~mC], idxD[~mC], dd[~mC], K2D,
                                 sdt=s2dt)

        dinvb = np.zeros((BLK, BPC), dtype=np.float32)
        nodes_at = np.full(RPAD, -1, dtype=np.int64)
        nodes_at[perms[c]] = np.arange(RPC)
        valid = nodes_at >= 0
        dv = np.zeros(RPAD, np.float32)
        dv[valid] = dinv[nodes_at[valid] + c * RPC]
        dinvb[:, :] = dv.reshape(BPC, BLK).T

        in_maps.append({
            "xT": xT, "w1": w1b, "w2": w2b, "ident": ident,
            "b1row": b1row, "b2row": b2row,
            "dinvb": dinvb,
            "i1a": i1a, "s2a": s2a, "i1b": i1b, "s2b": s2b,
            "i2c": i2c, "s2c": s2c, "i2d": i2d, "s2d": s2d,
        })
        real += len(s_c)
        padded += BLK * BPC * (K1A + K1B)

    LAST["K"] = (K1A, K1B, K2C, K2D)
    LAST["pad_frac"] = padded / real - 1.0
    return in_maps, perms, (K1A, K1B, K2C, K2D)


# ---------------- device program ----------------

def _build_program(K1A, K1B, K2C, K2D):
    dt = mybir.dt
    Z1DT = dt.float8e4 if USE_FP8 else dt.bfloat16
    S2DT = dt.float8e4 if USE_FP8S2 else dt.bfloat16
    phases = int(os.environ.get("GCN_PHASES", "3"))
    nc = bacc.Bacc(None, target_bir_lowering=False, debug=False,
                   num_devices=NCORES, num_swdge_queues=NQ,
                   dynamic_dma_scratch_size=int(
                       os.environ.get("GCN_SCRATCH", "16384")))

    xT = nc.dram_tensor("xT", [D_IN, NPAD], dt.bfloat16, kind="ExternalInput")
    w1 = nc.dram_tensor("w1", [D_IN, D_HID], dt.bfloat16, kind="ExternalInput")
    w2 = nc.dram_tensor("w2", [D_HID, D_OUT], dt.bfloat16, kind="ExternalInput")
    ident = nc.dram_tensor("ident", [128, 128], dt.bfloat16, kind="ExternalInput")
    b1row = nc.dram_tensor("b1row", [128, D_HID], Z1DT,
                           kind="ExternalInput")
    b2row = nc.dram_tensor("b2row", [16, 128], dt.bfloat16,
                           kind="ExternalInput")
    dinvb = nc.dram_tensor("dinvb", [128, BPC], dt.float32, kind="ExternalInput")

    def idx_t(name, K):
        return nc.dram_tensor(name, [128, BPC * K * BLK // 16], dt.int16,
                              kind="ExternalInput")

    def s2_t(name, K, sdt):
        return nc.dram_tensor(name, [128, BPC * K * BLK], sdt,
                              kind="ExternalInput")

    i1a, s2a = idx_t("i1a", K1A), s2_t("s2a", K1A, Z1DT)
    i1b, s2b = idx_t("i1b", K1B), s2_t("s2b", K1B, Z1DT)
    i2c, s2c = idx_t("i2c", K2C), s2_t("s2c", K2C, S2DT)
    i2d, s2d = idx_t("i2d", K2D), s2_t("s2d", K2D, S2DT)

    out = nc.dram_tensor("out", [RPAD, D_OUT], dt.float32, kind="ExternalOutput")

    z1A = nc.dram_tensor("z1A", [SPLIT1 + 128, D_HID], Z1DT)
    z1B = nc.dram_tensor("z1B", [NPAD - SPLIT1, D_HID], Z1DT)
    z2in0 = nc.dram_tensor("z2in0", [CPR, 128], dt.bfloat16)
    z2in1 = nc.dram_tensor("z2in1", [CPR, 128], dt.bfloat16)
    z2P0 = nc.dram_tensor("z2P0", [Z2ROWS, 128], dt.bfloat16,
                          addr_space="Shared")
    z2P1 = nc.dram_tensor("z2P1", [Z2ROWS, 128], dt.bfloat16,
                          addr_space="Shared")

    qctr = [0]

    def next_q():
        q = qctr[0] % NQ
        qctr[0] += 1
        return q

    with tile.TileContext(nc) as tc:
        with tc.tile_pool(name="consts", bufs=1) as cp, \
             tc.tile_pool(name="ph0x", bufs=2) as xp, \
             tc.tile_pool(name="ph0o", bufs=3) as op0, \
             tc.tile_pool(name="gp", bufs=4) as gp, \
             tc.tile_pool(name="csp", bufs=BPC + 1) as csp, \
             tc.tile_pool(name="s2p", bufs=2) as s2p, \
             tc.tile_pool(name="hp", bufs=2) as hp, \
             tc.tile_pool(name="zp", bufs=3) as zp, \
             tc.tile_pool(name="smp", bufs=4) as smp, \
             tc.tile_pool(name="psAcc", bufs=3, space="PSUM") as psAcc, \
             tc.tile_pool(name="psMisc", bufs=1, space="PSUM") as psMisc, \
             tc.tile_pool(name="psO", bufs=3, space="PSUM") as psO:
            w1t = cp.tile([128, 4, D_HID], dt.bfloat16)
            nc.sync.dma_start(
                w1t[:], w1.ap().rearrange("(k p) n -> p k n", p=128))
            w2t = cp.tile([128, 2, D_OUT], dt.bfloat16)
            nc.sync.dma_start(
                w2t[:], w2.ap().rearrange("(k p) n -> p k n", p=128))
            idt = cp.tile([128, 128], dt.bfloat16)
            nc.sync.dma_start(idt[:], ident[:, :])
            dvt = cp.tile([128, BPC], dt.float32)
            nc.sync.dma_start(dvt[:], dinvb[:, :])
            # bias rows into the gather tables / AG contributions
            nc.sync.dma_start(z1A.ap()[SPLIT1:SPLIT1 + 128, :], b1row.ap()[:, :])
            nc.sync.dma_start(z2in0.ap()[S0_ROWS:CPR, :], b2row.ap()[:, :])
            nc.sync.dma_start(z2in1.ap()[S0_ROWS:CPR, :], b2row.ap()[:, :])
            it1a = cp.tile([128, BPC * K1A * BLK // 16], dt.int16)
            nc.scalar.dma_start(it1a[:], i1a[:, :])
            it1b = cp.tile([128, BPC * K1B * BLK // 16], dt.int16)
            nc.scalar.dma_start(it1b[:], i1b[:, :])
            it2c = cp.tile([128, BPC * K2C * BLK // 16], dt.int16)
            nc.scalar.dma_start(it2c[:], i2c[:, :])
            it2d = cp.tile([128, BPC * K2D * BLK // 16], dt.int16)
            nc.scalar.dma_start(it2d[:], i2d[:, :])
            itabs = {(1, "A"): it1a, (1, "B"): it1b,
                     (2, "C"): it2c, (2, "D"): it2d}

            # ---------------- phases 1+2 stream tables ----------------
            seg1 = {
                "A": (K1A, i1a, s2a, z1A.ap()[:, :]),
                "B": (K1B, i1b, s2b, z1B.ap()[:, :]),
            }
            seg2 = {
                "C": (K2C, i2c, s2c, z2P0.ap()[:, :]),
                "D": (K2D, i2d, s2d, z2P1.ap()[:, :]),
            }
            gtiles = {}
            s2tiles = {}

            def ensure_g(layer, s, pi):
                key = (layer, s, pi)
                if key in gtiles:
                    return gtiles[key]
                K, idrm, s2drm, zview = (seg1 if layer == 1 else seg2)[s]
                felem = D_HID if layer == 1 else 128
                gdt = Z1DT if layer == 1 else dt.bfloat16
                SL = BPC * K * BLK
                n = min(PIECE, SL - pi * PIECE)
                off = pi * (PIECE // 16)
                it = itabs[(layer, s)]
                gt = gp.tile([128, PIECE // 128, felem], gdt,
                             tag=f"g{layer}{s}")
                nc.gpsimd.dma_gather(
                    gt[:, :n // 128, :], zview, it[:, off:off + n // 16],
                    n, n, felem, queue_num=next_q())
                gtiles[key] = gt
                return gt

            def ensure_s2(layer, s, pi):
                key = (layer, s, pi)
                if key in s2tiles:
                    return s2tiles[key]
                K, idrm, s2drm, zview = (seg1 if layer == 1 else seg2)[s]
                sdt = Z1DT if layer == 1 else S2DT
                nch = BPC * K
                n = min(S2CH, nch - pi * S2CH)
                st = s2p.tile([128, S2CH, 128], sdt, tag=f"s{layer}{s}")
                nc.scalar.dma_start(
                    st[:, :n, :],
                    s2drm.ap()[:, pi * S2CH * 128:(pi * S2CH + n) * 128]
                    .rearrange("p (n c) -> p n c", c=128))
                s2tiles[key] = st
                return st

            PCH = PIECE // BLK       # gather chunks per piece

            def l1_block(b):
                # psum accumulates sum_e dinv[d]*dinv[s]*z1[s] + b1 directly
                hps = psAcc.tile([128, D_HID], dt.float32, tag="acc")
                for s in ("A", "B"):
                    K = seg1[s][0]
                    k = 0
                    while k < K:
                        ci = b * K + k
                        gpi, gpos = divmod(ci * BLK, PIECE)
                        spi, spos = divmod(ci, S2CH)
                        cp_ = gpos // BLK
                        gt = ensure_g(1, s, gpi)
                        st = ensure_s2(1, s, spi)
                        pair = (USE_FP8 and k + 1 < K and cp_ + 1 < PCH
                                and spos + 1 < S2CH)
                        start = (s == "A" and k == 0)
                        if pair:
                            stop = (s == "B" and k + 2 == K)
                            nc.tensor.matmul(
                                hps[:],
                                st[:, spos:spos + 2, :],
                                gt[:, cp_:cp_ + 2, :],
                                start=start, stop=stop,
                                perf_mode=mybir.MatmulPerfMode.DoubleRow)
                            k += 2
                        else:
                            stop = (s == "B" and k + 1 == K)
                            nc.tensor.matmul(
                                hps[:],
                                st[:, spos, :],
                                gt[:, cp_, :],
                                start=start, stop=stop)
                            k += 1
                hr = hp.tile([128, D_HID], dt.bfloat16, tag="hr")
                nc.scalar.activation(
                    hr[:], hps[:], mybir.ActivationFunctionType.Relu)
                hT = hp.tile([128, 2, 128], dt.bfloat16, tag="hT")
                for h in range(2):
                    tps = psMisc.tile([128, 128], dt.bfloat16, tag="tps")
                    nc.tensor.transpose(
                        tps[:], hr[:, h * 128:(h + 1) * 128], idt[:])
                    nc.scalar.copy(hT[:, h, :], tps[:])
                zps = psMisc.tile([128, D_OUT], dt.float32, tag="zps")
                for h in range(2):
                    nc.tensor.matmul(
                        zps[:], hT[:, h, :], w2t[:, h, :],
                        start=(h == 0), stop=(h == 1))
                z2s = zp.tile([128, 128], dt.bfloat16, tag="z2s")
                nc.scalar.activation(
                    z2s[:, :D_OUT], zps[:],
                    mybir.ActivationFunctionType.Copy, scale=dvt[:, b:b + 1])
                if b < S0_BLOCKS:
                    nc.sync.dma_start(
                        z2in0.ap()[b * BLK:(b + 1) * BLK, :], z2s[:])
                else:
                    bb = b - S0_BLOCKS
                    nc.sync.dma_start(
                        z2in1.ap()[bb * BLK:(bb + 1) * BLK, :], z2s[:])

            cstash = {}

            def l2cd_block(b, s):
                ops = psO.tile([128, D_OUT], dt.float32, tag="ops")
                K = seg2[s][0]
                for k in range(K):
                    ci = b * K + k
                    gpi, gpos = divmod(ci * BLK, PIECE)
                    spi, spos = divmod(ci, S2CH)
                    gt = ensure_g(2, s, gpi)
                    st = ensure_s2(2, s, spi)
                    nc.tensor.matmul(
                        ops[:],
                        st[:, spos, :],
                        gt[:, (gpos // BLK), :D_OUT],
                        start=(k == 0), stop=(k == K - 1))
                return ops

            def l2c_block(b):
                ops = l2cd_block(b, "C")
                cs = csp.tile([128, D_OUT], dt.float32, tag="cs")
                nc.scalar.copy(cs[:], ops[:])
                cstash[b] = cs

            def l2d_block(b):
                ops = l2cd_block(b, "D")
                t2 = smp.tile([128, D_OUT], dt.float32, tag="t2")
                nc.vector.tensor_tensor(
                    t2[:], ops[:], cstash[b][:], op=mybir.AluOpType.add)
                nm = smp.tile([128, 1], dt.float32, tag="nm")
                nc.vector.reduce_max(
                    nm[:], t2[:], axis=mybir.AxisListType.X, negate=True)
                ex = smp.tile([128, D_OUT], dt.float32, tag="ex")
                sm = smp.tile([128, 1], dt.float32, tag="sm")
                nc.scalar.activation(
                    ex[:], t2[:], mybir.ActivationFunctionType.Exp,
                    bias=nm[:], accum_out=sm[:])
                rc = smp.tile([128, 1], dt.float32, tag="rc")
                nc.vector.reciprocal(rc[:], sm[:])
                ot = smp.tile([128, D_OUT], dt.float32, tag="ot")
                nc.vector.tensor_scalar(
                    ot[:], ex[:], rc[:], None, op0=mybir.AluOpType.mult)
                nc.sync.dma_start(out.ap()[b * BLK:(b + 1) * BLK, :], ot[:])

            # ---------------- phase 0: z1 = xT^T @ W1 (A half then B half) ---
            z1Av = z1A.ap()[0:SPLIT1, :].rearrange("(n p) f -> p n f", p=128)
            z1Bv = z1B.ap().rearrange("(n p) f -> p n f", p=128)
            NB_A = SPLIT1 // BLK
            GB = 7
            GRP = 16

            def phase0_range(glo, ghi):
                for g0 in range(glo, ghi, GRP):
                    gb = min(GRP, NBLOCKS - g0)
                    if gb <= 0:
                        break
                    xg = xp.tile([128, 4, GRP * BLK], dt.bfloat16, tag="xg")
                    nc.sync.dma_start(
                        xg[:, :, :gb * BLK],
                        xT.ap().rearrange("(k p) n -> p k n", p=128)
                        [:, :, g0 * BLK:(g0 + gb) * BLK])
                    for b0 in range(0, gb, GB):
                        nb = min(GB, gb - b0)
                        zo = op0.tile([128, GB, D_HID], Z1DT, tag="zo")
                        for i in range(nb):
                            ps = psAcc.tile([128, D_HID], dt.float32, tag="acc")
                            col = (b0 + i) * BLK
                            for k in range(4):
                                nc.tensor.matmul(
                                    ps[:],
                                    xg[:, k, col:col + BLK],
                                    w1t[:, k, :],
                                    start=(k == 0), stop=(k == 3))
                            nc.vector.tensor_copy(zo[:, i, :], ps[:])
                        lo, hi = g0 + b0, g0 + b0 + nb
                        if hi <= NB_A:
                            nc.sync.dma_start(z1Av[:, lo:hi, :], zo[:, :nb, :])
                        elif lo >= NB_A:
                            nc.sync.dma_start(
                                z1Bv[:, lo - NB_A:hi - NB_A, :], zo[:, :nb, :])
                        else:
                            na = NB_A - lo
                            nc.sync.dma_start(z1Av[:, lo:NB_A, :], zo[:, :na, :])
                            nc.sync.dma_start(
                                z1Bv[:, 0:hi - NB_A, :], zo[:, na:nb, :])

            pref = int(os.environ.get("GCN_PREF", "1"))
            nl1 = int(os.environ.get("GCN_L1BLOCKS", str(BPC)))
            if phases >= 1:
                # phase 0 A-half (z1A rows), then prefetch the first L1-A
                # gather pieces so SWDGE ramps while phase 0 B computes.
                phase0_range(0, 200)
                if pref:
                    ensure_s2(1, "A", 0)
                    for pi in range(4):
                        ensure_g(1, "A", pi)
                phase0_range(208, NBLOCKS)

                for b in range(min(S0_BLOCKS, nl1)):
                    l1_block(b)
                if phases >= 2:
                    nc.gpsimd.collective_compute(
                        "AllGather", mybir.AluOpType.bypass,
                        replica_groups=[list(range(NCORES))],
                        ins=[z2in0.ap().opt()],
                        outs=[z2P0.ap().opt()])
                ci = 0
                for b in range(S0_BLOCKS, min(BPC, nl1)):
                    l1_block(b)
                    if phases >= 3 and b >= S0_BLOCKS + 12 and ci < BPC:
                        l2c_block(ci)
                        ci += 1
                if phases >= 2:
                    nc.gpsimd.collective_compute(
                        "AllGather", mybir.AluOpType.bypass,
                        replica_groups=[list(range(NCORES))],
                        ins=[z2in1.ap().opt()],
                        outs=[z2P1.ap().opt()])
                if phases >= 3:
                    while ci < BPC:
                        l2c_block(ci)
                        ci += 1
                    for b in range(BPC):
                        l2d_block(b)

    nc.compile()
    return nc


# ---------------- entry point ----------------

def kernel(x, edge_index, W1, b1, W2, b2):
    x = np.asarray(x)
    edge_index = np.asarray(edge_index)
    in_maps, perms, Ks = _preprocess(x, edge_index, W1, b1, W2, b2)
    nc = _build_program(*Ks)

    trace = os.environ.get("GCN_TRACE", "0") == "1"
    if trace:
        trace = _install_trace_hook()
    res = run_bass_kernel_spmd(
        nc, in_maps, core_ids=list(range(NCORES)), trace=trace)
    LAST["exec_time_ns"] = res.exec_time_ns
    LAST["results"] = res

    out = np.empty((N_NODES, D_OUT), dtype=np.float32)
    for c in range(NCORES):
        oc = np.asarray(res.results[c]["out"], dtype=np.float32)
        out[c * RPC:(c + 1) * RPC] = oc[perms[c]]
    return out


# revision 47
# speedup vs baseline: 1.9986x; 1.0106x over previous
"""GCN (2-layer, PyG GCNConv semantics) on 8 Trainium2 NeuronCores.

Strategy (dst-sharded message passing):
  out = softmax( A @ relu(A @ (x W1) + b1) @ W2 + b2 ),  A = D^-1/2 (Adj+I) D^-1/2

  - Host: degrees/dinv, self-loops appended as ordinary edges, edges
    partitioned by destination core (6250 dst rows per core), each core's
    dst nodes permuted into 50 load-balanced blocks of 128.  Per-edge
    gather indices (int16) and one-hot segment-sum matrices (bf16, with
    dinv[dst] folded in; bias rows folded in as extra "edges") are
    precomputed on the host and streamed to the device.
  - Phase 0 (on-device, redundant per core): z1 = (dinv*x) @ W1 in bf16,
    stored to local HBM (the layer-1 gather table).
  - Phase 1: per-edge dma_gather of z1 rows (4 SWDGE queues round-robin,
    4096-row pieces); segment-sum via TensorE matmuls h += S^T @ G
    (S = one-hot with dinv[dst]); relu on ScalarE; z2 = dinv * (h @ W2).
  - AllGather of z2 (bf16, rows padded to 128 cols) across the 8 cores
    in two row-slices.
  - Phase 2: per-edge dma_gather of z2 rows (bf16 256B rows), segment-sum
    to output blocks, softmax, DMA out.

kernel(**inputs) -> np.ndarray is self-contained (shapes hardcoded).
"""

import os
import sys
import types

sys.path.insert(0, "/opt/trn_rl_repo")

import numpy as np
import ml_dtypes

from concourse import bass, mybir, bacc, tile
from concourse.bass_utils import run_bass_kernel_spmd

BF16 = ml_dtypes.bfloat16
FP8 = ml_dtypes.float8_e4m3   # matches mybir dt.float8e4 (TRN FP8_EXP4)
USE_FP8 = os.environ.get("GCN_FP8", "1") == "1"      # z1 table + L1 one-hots
USE_FP8S2 = os.environ.get("GCN_FP8S2", "1") == "1"  # L2 one-hots (mixed mm)
USE_FP8X = os.environ.get("GCN_FP8X", "1") == "1"    # x / W1 (phase 0)

# ---------------- problem constants (hardcoded) ----------------
N_NODES = 50000
D_IN, D_HID, D_OUT = 512, 256, 64
NCORES = 8
RPC = N_NODES // NCORES          # 6250 dst rows per core
BLK = 128
BPC = 50                         # blocks per core (spare slots for balancing)
RPAD = BPC * BLK                 # 6400
NPAD = ((N_NODES + BLK - 1) // BLK) * BLK   # 50048 (391 node blocks)
NBLOCKS = NPAD // BLK            # 391
SPLIT1 = 24960                   # L1 gather src split (block-aligned, int16-safe)
S0_ROWS = 3200                   # AG slice 0: perm positions [0, 3200) = 25 blocks
S1_ROWS = RPAD - S0_ROWS         # 3200: positions [3200, 6400) = 25 blocks
S0_BLOCKS = S0_ROWS // BLK       # 25
CPR = S0_ROWS + 16               # rows per AG contribution (16 = b2 bias pad)
Z2ROWS = NCORES * CPR            # 25728 rows per z2 table
PIECE = int(os.environ.get("GCN_PIECE", "1024"))   # slots per dma_gather
                                 # (>1024 overflows the SWDGE ring: hangs)
S2CH = 32                        # one-hot chunks per S2 stream DMA piece
NQ = 4                           # SWDGE queues (ucode max)

LAST = {}                        # test harness introspection


def _install_trace_hook():
    try:
        mod = types.ModuleType("antenv.axon_hooks")
        hook = [None]
        mod.set_axon_ntff_profile_hook = lambda h: hook.__setitem__(0, h)
        mod.get_axon_ntff_profile_hook = lambda: hook[0]
        sys.modules["antenv.axon_hooks"] = mod
        import antenv
        antenv.axon_hooks = mod
        from trn_agent_boot.trn_boot import _ntff_profile_via_ctypes
        mod.set_axon_ntff_profile_hook(
            _ntff_profile_via_ctypes("/opt/axon/libaxon_pjrt.so"))
        return True
    except Exception:
        return False


# ---------------- host-side preprocessing ----------------

def _pack_greedy(node_ids, cnts, block_ids, cap):
    """Greedy k-dim balanced packing of node_ids into block_ids (<=128 each).
    cnts: [ndim, RPC] per-node counts. Returns {node: block}."""
    nd = len(cnts)
    nb = len(block_ids)
    tot = sum(c[node_ids] for c in cnts)
    order = node_ids[np.argsort(-tot, kind="stable")]
    sums = np.zeros((nd, nb), dtype=np.float64)
    cnt = np.zeros(nb, dtype=np.int64)
    assign = {}
    big = 1e18
    for i in order:
        score = np.max([(sums[d] + cnts[d][i]) / cap for d in range(nd)], axis=0)
        score = score + (sums.sum(axis=0) + tot[0] * 0) * 1e-7
        score = np.where(cnt < BLK, score, big)
        j = int(np.argmin(score))
        assign[i] = j
        cnt[j] += 1
        for d in range(nd):
            sums[d, j] += cnts[d][i]
    # repair per dim
    members = {j: [i for i, jj in assign.items() if jj == j] for j in range(nb)}
    for d in range(nd):
        for _ in range(2000):
            j = int(np.argmax(sums[d]))
            if sums[d, j] <= cap:
                break
            ms = members[j]
            pos_m = [i for i in ms if cnts[d][i] > 0]
            if not pos_m:
                break
            mv = min(pos_m, key=lambda i: cnts[d][i])
            tgt = np.where(cnt < BLK, sums[d], big)
            tgt[j] = big
            jt = int(np.argmin(tgt))
            if tgt[jt] >= big:
                break
            assign[mv] = jt
            members[j].remove(mv)
            members[jt].append(mv)
            cnt[j] -= 1
            cnt[jt] += 1
            for dd in range(nd):
                sums[dd, j] -= cnts[dd][mv]
                sums[dd, jt] += cnts[dd][mv]
    return assign


def _positions_from_assign(assign, block_ids):
    pos = {}
    slot = {j: 0 for j in block_ids}
    for i in sorted(assign):
        j = assign[i]
        pos[i] = j * BLK + slot[j]
        slot[j] += 1
    return pos


def _pack_blocks(cntA, cntB, cap=1148):
    nodes = np.arange(RPC)
    assign = _pack_greedy(nodes, [cntA, cntB], list(range(BPC)), cap)
    posd = _positions_from_assign(assign, list(range(BPC)))
    pos = np.empty(RPC, dtype=np.int64)
    for i in range(RPC):
        pos[i] = posd[i]
    return pos


def _pack_blocks4(cntA, cntB, cntC, cntD, half0_nodes, cap=1148):
    """Second pass: rebalance within halves on 4 dims."""
    pos = np.empty(RPC, dtype=np.int64)
    all_nodes = np.arange(RPC)
    h0 = half0_nodes
    h1 = all_nodes[~np.isin(all_nodes, h0)]
    for nodes, blocks in ((h0, list(range(S0_BLOCKS))),
                          (h1, list(range(S0_BLOCKS, BPC)))):
        assign = _pack_greedy(nodes, [cntA, cntB, cntC, cntD], blocks, cap)
        # blocks list indexes into _pack_greedy's local 0..nb-1 space
        posd = {}
        slot = {j: 0 for j in range(len(blocks))}
        for i in sorted(assign):
            j = assign[i]
            posd[i] = blocks[j] * BLK + slot[j]
            slot[j] += 1
        for i in nodes:
            pos[i] = posd[i]
    return pos


def _build_stream(e_pos, e_idx16, e_dd, K, bias_idx=None, sdt=BF16):
    """Returns (idx_wrapped [128, SL/16] i16, s2 [128, nch*128] sdt).
    e_dd: per-edge weight folded into the one-hot matrix (dinv[dst]).
    bias_idx: if set, one extra slot per block gathers this row and adds it
    (weight 1.0) to every dst position of the block (bias fold-in)."""
    nch = BPC * K
    SL = nch * BLK
    blk = e_pos // BLK
    o = np.argsort(blk, kind="stable")
    blk_s = blk[o]
    e_pos = e_pos[o]
    e_idx16 = e_idx16[o]
    dd = e_dd[o] if e_dd is not None else np.ones(len(o), np.float32)
    counts = np.bincount(blk_s, minlength=BPC)
    cap = K * BLK - (1 if bias_idx is not None else 0)
    assert counts.max() <= cap, (counts.max(), cap)
    starts = np.concatenate([[0], np.cumsum(counts)[:-1]])
    within = np.arange(len(blk_s)) - np.repeat(starts, counts)
    slot = blk_s * (K * BLK) + within

    idx_full = np.zeros(SL, dtype=np.int16)
    idx_full[slot] = e_idx16
    s2 = np.zeros((128, nch, 128), dtype=sdt)
    s2[slot % BLK, slot // BLK, (e_pos % BLK)] = dd.astype(sdt)
    if bias_idx is not None:
        for b in range(BPC):
            fs = b * (K * BLK) + counts[b]     # first free slot of block b
            idx_full[fs] = bias_idx
            s2[fs % BLK, fs // BLK, :] = np.ones(128, dtype=sdt)
    idx_w = np.tile(idx_full.reshape(SL // 16, 16).T, (8, 1)).copy()
    return idx_w, s2.reshape(128, nch * 128)


def _preprocess(x, edge_index, W1, b1, W2, b2):
    src = np.asarray(edge_index[0], dtype=np.int64)
    dst = np.asarray(edge_index[1], dtype=np.int64)
    loops = np.arange(N_NODES, dtype=np.int64)
    src_all = np.concatenate([src, loops])
    dst_all = np.concatenate([dst, loops])
    deg = np.bincount(dst_all, minlength=N_NODES).astype(np.float32)
    dinv = np.where(deg > 0, 1.0 / np.sqrt(deg), 0.0).astype(np.float32)

    core_of = dst_all // RPC

    perms = []
    core_edges = []
    cnts_ab = []
    for c in range(NCORES):
        m = core_of == c
        s_c = src_all[m]
        d_loc = (dst_all[m] - c * RPC).astype(np.int64)
        cntA = np.bincount(d_loc[s_c < SPLIT1], minlength=RPC)
        cntB = np.bincount(d_loc[s_c >= SPLIT1], minlength=RPC)
        perms.append(_pack_blocks(cntA, cntB))
        core_edges.append((s_c, d_loc))
        cnts_ab.append((cntA, cntB))

    permpos_global = np.empty(N_NODES, dtype=np.int64)
    for c in range(NCORES):
        permpos_global[c * RPC:(c + 1) * RPC] = perms[c]

    # pass 2: rebalance within halves, also evening C/D (src-half) counts
    half_global = permpos_global < S0_ROWS
    perms2 = []
    for c in range(NCORES):
        s_c, d_loc = core_edges[c]
        cntA, cntB = cnts_ab[c]
        hsrc = half_global[s_c]
        cntC = np.bincount(d_loc[hsrc], minlength=RPC)
        cntD = np.bincount(d_loc[~hsrc], minlength=RPC)
        half0_nodes = np.where(perms[c] < S0_ROWS)[0]
        perms2.append(_pack_blocks4(cntA, cntB, cntC, cntD, half0_nodes))
    perms = perms2
    for c in range(NCORES):
        permpos_global[c * RPC:(c + 1) * RPC] = perms[c]

    def seg_K(e_pos, extra=0):
        counts = np.bincount(e_pos // BLK, minlength=BPC)
        return int(np.ceil((counts.max() + extra) / BLK))

    K1A = K1B = K2C = K2D = 1
    meta = []
    for c in range(NCORES):
        s_c, d_loc = core_edges[c]
        pos_d = perms[c][d_loc]
        mA = s_c < SPLIT1
        src_r = s_c // RPC
        src_pos = permpos_global[s_c]   # core-local position (0..RPAD-1)
        mC = src_pos < S0_ROWS
        K1A = max(K1A, seg_K(pos_d[mA], 1))
        K1B = max(K1B, seg_K(pos_d[~mA]))
        K2C = max(K2C, seg_K(pos_d[mC], 1))
        K2D = max(K2D, seg_K(pos_d[~mC]))
        meta.append((s_c, d_loc, pos_d, mA, mC, src_r, src_pos))

    in_maps = []
    xdt = FP8 if USE_FP8X else BF16
    xs = (np.asarray(x, np.float32) * dinv[:, None])
    xT = np.zeros((D_IN, NPAD), dtype=xdt)
    xT[:, :N_NODES] = xs.T.astype(xdt)
    w1b = np.asarray(W1, np.float32).astype(xdt)
    w2b = np.asarray(W2, np.float32).astype(BF16)
    ident = np.eye(128, dtype=np.float32).astype(BF16)
    # bias rows: b1 as a gatherable z1-table row, b2 as a z2-table row
    b1row = np.zeros((128, D_HID), dtype=FP8 if USE_FP8 else BF16)
    b1row[0, :] = np.asarray(b1, np.float32).astype(b1row.dtype)
    b2row = np.zeros((16, 128), dtype=BF16)
    b2row[0, :D_OUT] = np.asarray(b2, np.float32).astype(BF16)

    real = padded = 0
    for c in range(NCORES):
        s_c, d_loc, pos_d, mA, mC, src_r, src_pos = meta[c]
        dd = dinv[d_loc + c * RPC]    # dinv[dst] per edge
        s1dt = FP8 if USE_FP8 else BF16
        s2dt = FP8 if USE_FP8S2 else BF16
        i1a, s2a = _build_stream(pos_d[mA], s_c[mA].astype(np.int16),
                                 dd[mA], K1A, bias_idx=SPLIT1, sdt=s1dt)
        i1b, s2b = _build_stream(pos_d[~mA],
                                 (s_c[~mA] - SPLIT1).astype(np.int16),
                                 dd[~mA], K1B, sdt=s1dt)
        idxC = (src_r * CPR + src_pos).astype(np.int16)
        idxD = (src_r * CPR + (src_pos - S0_ROWS)).astype(np.int16)
        i2c, s2c = _build_stream(pos_d[mC], idxC[mC], dd[mC], K2C,
                                 bias_idx=S0_ROWS, sdt=s2dt)
        i2d, s2d = _build_stream(pos_d[~mC], idxD[~mC], dd[~mC], K2D,
                                 sdt=s2dt)

        dinvb = np.zeros((BLK, BPC), dtype=np.float32)
        nodes_at = np.full(RPAD, -1, dtype=np.int64)
        nodes_at[perms[c]] = np.arange(RPC)
        valid = nodes_at >= 0
        dv = np.zeros(RPAD, np.float32)
        dv[valid] = dinv[nodes_at[valid] + c * RPC]
        dinvb[:, :] = dv.reshape(BPC, BLK).T

        in_maps.append({
            "xT": xT, "w1": w1b, "w2": w2b, "ident": ident,
            "b1row": b1row, "b2row": b2row,
            "dinvb": dinvb,
            "i1a": i1a, "s2a": s2a, "i1b": i1b, "s2b": s2b,
            "i2c": i2c, "s2c": s2c, "i2d": i2d, "s2d": s2d,
        })
        real += len(s_c)
        padded += BLK * BPC * (K1A + K1B)

    LAST["K"] = (K1A, K1B, K2C, K2D)
    LAST["pad_frac"] = padded / real - 1.0
    return in_maps, perms, (K1A, K1B, K2C, K2D)


# ---------------- device program ----------------

def _build_program(K1A, K1B, K2C, K2D):
    dt = mybir.dt
    Z1DT = dt.float8e4 if USE_FP8 else dt.bfloat16
    S2DT = dt.float8e4 if USE_FP8S2 else dt.bfloat16
    XDT = dt.float8e4 if USE_FP8X else dt.bfloat16
    phases = int(os.environ.get("GCN_PHASES", "3"))
    nc = bacc.Bacc(None, target_bir_lowering=False, debug=False,
                   num_devices=NCORES, num_swdge_queues=NQ,
                   dynamic_dma_scratch_size=int(
                       os.environ.get("GCN_SCRATCH", "16384")))

    xT = nc.dram_tensor("xT", [D_IN, NPAD], XDT, kind="ExternalInput")
    w1 = nc.dram_tensor("w1", [D_IN, D_HID], XDT, kind="ExternalInput")
    w2 = nc.dram_tensor("w2", [D_HID, D_OUT], dt.bfloat16, kind="ExternalInput")
    ident = nc.dram_tensor("ident", [128, 128], dt.bfloat16, kind="ExternalInput")
    b1row = nc.dram_tensor("b1row", [128, D_HID], Z1DT,
                           kind="ExternalInput")
    b2row = nc.dram_tensor("b2row", [16, 128], dt.bfloat16,
                           kind="ExternalInput")
    dinvb = nc.dram_tensor("dinvb", [128, BPC], dt.float32, kind="ExternalInput")

    def idx_t(name, K):
        return nc.dram_tensor(name, [128, BPC * K * BLK // 16], dt.int16,
                              kind="ExternalInput")

    def s2_t(name, K, sdt):
        return nc.dram_tensor(name, [128, BPC * K * BLK], sdt,
                              kind="ExternalInput")

    i1a, s2a = idx_t("i1a", K1A), s2_t("s2a", K1A, Z1DT)
    i1b, s2b = idx_t("i1b", K1B), s2_t("s2b", K1B, Z1DT)
    i2c, s2c = idx_t("i2c", K2C), s2_t("s2c", K2C, S2DT)
    i2d, s2d = idx_t("i2d", K2D), s2_t("s2d", K2D, S2DT)

    out = nc.dram_tensor("out", [RPAD, D_OUT], dt.float32, kind="ExternalOutput")

    z1A = nc.dram_tensor("z1A", [SPLIT1 + 128, D_HID], Z1DT)
    z1B = nc.dram_tensor("z1B", [NPAD - SPLIT1, D_HID], Z1DT)
    z2in0 = nc.dram_tensor("z2in0", [CPR, 128], dt.bfloat16)
    z2in1 = nc.dram_tensor("z2in1", [CPR, 128], dt.bfloat16)
    z2P0 = nc.dram_tensor("z2P0", [Z2ROWS, 128], dt.bfloat16,
                          addr_space="Shared")
    z2P1 = nc.dram_tensor("z2P1", [Z2ROWS, 128], dt.bfloat16,
                          addr_space="Shared")

    qctr = [0]

    def next_q():
        q = qctr[0] % NQ
        qctr[0] += 1
        return q

    with tile.TileContext(nc) as tc:
        with tc.tile_pool(name="consts", bufs=1) as cp, \
             tc.tile_pool(name="ph0x", bufs=2) as xp, \
             tc.tile_pool(name="ph0o", bufs=3) as op0, \
             tc.tile_pool(name="gp", bufs=4) as gp, \
             tc.tile_pool(name="csp", bufs=BPC + 1) as csp, \
             tc.tile_pool(name="s2p", bufs=2) as s2p, \
             tc.tile_pool(name="hp", bufs=2) as hp, \
             tc.tile_pool(name="zp", bufs=3) as zp, \
             tc.tile_pool(name="smp", bufs=8) as smp, \
             tc.tile_pool(name="psAcc", bufs=3, space="PSUM") as psAcc, \
             tc.tile_pool(name="psMisc", bufs=1, space="PSUM") as psMisc, \
             tc.tile_pool(name="psO", bufs=3, space="PSUM") as psO:
            w1t = cp.tile([128, 4, D_HID], XDT)
            nc.sync.dma_start(
                w1t[:], w1.ap().rearrange("(k p) n -> p k n", p=128))
            w2t = cp.tile([128, 2, D_OUT], dt.bfloat16)
            nc.sync.dma_start(
                w2t[:], w2.ap().rearrange("(k p) n -> p k n", p=128))
            idt = cp.tile([128, 128], dt.bfloat16)
            nc.sync.dma_start(idt[:], ident[:, :])
            dvt = cp.tile([128, BPC], dt.float32)
            nc.sync.dma_start(dvt[:], dinvb[:, :])
            # bias rows into the gather tables / AG contributions
            nc.sync.dma_start(z1A.ap()[SPLIT1:SPLIT1 + 128, :], b1row.ap()[:, :])
            nc.sync.dma_start(z2in0.ap()[S0_ROWS:CPR, :], b2row.ap()[:, :])
            nc.sync.dma_start(z2in1.ap()[S0_ROWS:CPR, :], b2row.ap()[:, :])
            it1a = cp.tile([128, BPC * K1A * BLK // 16], dt.int16)
            nc.scalar.dma_start(it1a[:], i1a[:, :])
            it1b = cp.tile([128, BPC * K1B * BLK // 16], dt.int16)
            nc.scalar.dma_start(it1b[:], i1b[:, :])
            it2c = cp.tile([128, BPC * K2C * BLK // 16], dt.int16)
            nc.scalar.dma_start(it2c[:], i2c[:, :])
            it2d = cp.tile([128, BPC * K2D * BLK // 16], dt.int16)
            nc.scalar.dma_start(it2d[:], i2d[:, :])
            itabs = {(1, "A"): it1a, (1, "B"): it1b,
                     (2, "C"): it2c, (2, "D"): it2d}

            # ---------------- phases 1+2 stream tables ----------------
            seg1 = {
                "A": (K1A, i1a, s2a, z1A.ap()[:, :]),
                "B": (K1B, i1b, s2b, z1B.ap()[:, :]),
            }
            seg2 = {
                "C": (K2C, i2c, s2c, z2P0.ap()[:, :]),
                "D": (K2D, i2d, s2d, z2P1.ap()[:, :]),
            }
            gtiles = {}
            s2tiles = {}

            def ensure_g(layer, s, pi):
                key = (layer, s, pi)
                if key in gtiles:
                    return gtiles[key]
                K, idrm, s2drm, zview = (seg1 if layer == 1 else seg2)[s]
                felem = D_HID if layer == 1 else 128
                gdt = Z1DT if layer == 1 else dt.bfloat16
                SL = BPC * K * BLK
                n = min(PIECE, SL - pi * PIECE)
                off = pi * (PIECE // 16)
                it = itabs[(layer, s)]
                gt = gp.tile([128, PIECE // 128, felem], gdt,
                             tag=f"g{layer}{s}",
                             bufs=(12 if layer == 1 else 6))
                nc.gpsimd.dma_gather(
                    gt[:, :n // 128, :], zview, it[:, off:off + n // 16],
                    n, n, felem, queue_num=next_q())
                gtiles[key] = gt
                return gt

            def ensure_s2(layer, s, pi):
                key = (layer, s, pi)
                if key in s2tiles:
                    return s2tiles[key]
                K, idrm, s2drm, zview = (seg1 if layer == 1 else seg2)[s]
                sdt = Z1DT if layer == 1 else S2DT
                nch = BPC * K
                n = min(S2CH, nch - pi * S2CH)
                st = s2p.tile([128, S2CH, 128], sdt, tag=f"s{layer}{s}")
                nc.scalar.dma_start(
                    st[:, :n, :],
                    s2drm.ap()[:, pi * S2CH * 128:(pi * S2CH + n) * 128]
                    .rearrange("p (n c) -> p n c", c=128))
                s2tiles[key] = st
                return st

            PCH = PIECE // BLK       # gather chunks per piece

            def l1_block(b):
                # psum accumulates sum_e dinv[d]*dinv[s]*z1[s] + b1 directly
                hps = psAcc.tile([128, D_HID], dt.float32, tag="acc")
                for s in ("A", "B"):
                    K = seg1[s][0]
                    k = 0
                    while k < K:
                        ci = b * K + k
                        gpi, gpos = divmod(ci * BLK, PIECE)
                        spi, spos = divmod(ci, S2CH)
                        cp_ = gpos // BLK
                        gt = ensure_g(1, s, gpi)
                        st = ensure_s2(1, s, spi)
                        pair = (USE_FP8 and k + 1 < K and cp_ + 1 < PCH
                                and spos + 1 < S2CH)
                        start = (s == "A" and k == 0)
                        if pair:
                            stop = (s == "B" and k + 2 == K)
                            nc.tensor.matmul(
                                hps[:],
                                st[:, spos:spos + 2, :],
                                gt[:, cp_:cp_ + 2, :],
                                start=start, stop=stop,
                                perf_mode=mybir.MatmulPerfMode.DoubleRow)
                            k += 2
                        else:
                            stop = (s == "B" and k + 1 == K)
                            nc.tensor.matmul(
                                hps[:],
                                st[:, spos, :],
                                gt[:, cp_, :],
                                start=start, stop=stop)
                            k += 1
                hr = hp.tile([128, D_HID], dt.bfloat16, tag="hr")
                nc.scalar.activation(
                    hr[:], hps[:], mybir.ActivationFunctionType.Relu)
                hT = hp.tile([128, 2, 128], dt.bfloat16, tag="hT")
                for h in range(2):
                    tps = psMisc.tile([128, 128], dt.bfloat16, tag="tps")
                    nc.tensor.transpose(
                        tps[:], hr[:, h * 128:(h + 1) * 128], idt[:])
                    nc.scalar.copy(hT[:, h, :], tps[:])
                zps = psMisc.tile([128, D_OUT], dt.float32, tag="zps")
                for h in range(2):
                    nc.tensor.matmul(
                        zps[:], hT[:, h, :], w2t[:, h, :],
                        start=(h == 0), stop=(h == 1))
                z2s = zp.tile([128, 128], dt.bfloat16, tag="z2s")
                nc.scalar.activation(
                    z2s[:, :D_OUT], zps[:],
                    mybir.ActivationFunctionType.Copy, scale=dvt[:, b:b + 1])
                if b < S0_BLOCKS:
                    nc.sync.dma_start(
                        z2in0.ap()[b * BLK:(b + 1) * BLK, :], z2s[:])
                else:
                    bb = b - S0_BLOCKS
                    nc.sync.dma_start(
                        z2in1.ap()[bb * BLK:(bb + 1) * BLK, :], z2s[:])

            cstash = {}

            def l2cd_block(b, s):
                ops = psO.tile([128, D_OUT], dt.float32, tag="ops")
                K = seg2[s][0]
                for k in range(K):
                    ci = b * K + k
                    gpi, gpos = divmod(ci * BLK, PIECE)
                    spi, spos = divmod(ci, S2CH)
                    gt = ensure_g(2, s, gpi)
                    st = ensure_s2(2, s, spi)
                    nc.tensor.matmul(
                        ops[:],
                        st[:, spos, :],
                        gt[:, (gpos // BLK), :D_OUT],
                        start=(k == 0), stop=(k == K - 1))
                return ops

            def l2c_block(b):
                ops = l2cd_block(b, "C")
                cs = csp.tile([128, D_OUT], dt.float32, tag="cs")
                nc.scalar.copy(cs[:], ops[:])
                cstash[b] = cs

            def l2d_block(b):
                ops = l2cd_block(b, "D")
                t2 = smp.tile([128, D_OUT], dt.float32, tag="t2")
                nc.vector.tensor_tensor(
                    t2[:], ops[:], cstash[b][:], op=mybir.AluOpType.add)
                nm = smp.tile([128, 1], dt.float32, tag="nm")
                nc.vector.reduce_max(
                    nm[:], t2[:], axis=mybir.AxisListType.X, negate=True)
                ex = smp.tile([128, D_OUT], dt.float32, tag="ex")
                sm = smp.tile([128, 1], dt.float32, tag="sm")
                nc.scalar.activation(
                    ex[:], t2[:], mybir.ActivationFunctionType.Exp,
                    bias=nm[:], accum_out=sm[:])
                rc = smp.tile([128, 1], dt.float32, tag="rc")
                nc.vector.reciprocal(rc[:], sm[:])
                ot = smp.tile([128, D_OUT], dt.float32, tag="ot")
                nc.vector.tensor_scalar(
                    ot[:], ex[:], rc[:], None, op0=mybir.AluOpType.mult)
                nc.sync.dma_start(out.ap()[b * BLK:(b + 1) * BLK, :], ot[:])

            # ---------------- phase 0: z1 = xT^T @ W1 (A half then B half) ---
            z1Av = z1A.ap()[0:SPLIT1, :].rearrange("(n p) f -> p n f", p=128)
            z1Bv = z1B.ap().rearrange("(n p) f -> p n f", p=128)
            NB_A = SPLIT1 // BLK
            GB = 7
            GRP = 16

            def phase0_range(glo, ghi):
                for g0 in range(glo, ghi, GRP):
                    gb = min(GRP, NBLOCKS - g0)
                    if gb <= 0:
                        break
                    xg = xp.tile([128, 4, GRP * BLK], XDT, tag="xg")
                    nc.sync.dma_start(
                        xg[:, :, :gb * BLK],
                        xT.ap().rearrange("(k p) n -> p k n", p=128)
                        [:, :, g0 * BLK:(g0 + gb) * BLK])
                    for b0 in range(0, gb, GB):
                        nb = min(GB, gb - b0)
                        zo = op0.tile([128, GB, D_HID], Z1DT, tag="zo")
                        for i in range(nb):
                            ps = psAcc.tile([128, D_HID], dt.float32, tag="acc")
                            col = (b0 + i) * BLK
                            if USE_FP8X:
                                for k in (0, 2):
                                    nc.tensor.matmul(
                                        ps[:],
                                        xg[:, k:k + 2, col:col + BLK],
                                        w1t[:, k:k + 2, :],
                                        start=(k == 0), stop=(k == 2),
                                        perf_mode=mybir.MatmulPerfMode
                                        .DoubleRow)
                            else:
                                for k in range(4):
                                    nc.tensor.matmul(
                                        ps[:],
                                        xg[:, k, col:col + BLK],
                                        w1t[:, k, :],
                                        start=(k == 0), stop=(k == 3))
                            nc.vector.tensor_copy(zo[:, i, :], ps[:])
                        lo, hi = g0 + b0, g0 + b0 + nb
                        if hi <= NB_A:
                            nc.sync.dma_start(z1Av[:, lo:hi, :], zo[:, :nb, :])
                        elif lo >= NB_A:
                            nc.sync.dma_start(
                                z1Bv[:, lo - NB_A:hi - NB_A, :], zo[:, :nb, :])
                        else:
                            na = NB_A - lo
                            nc.sync.dma_start(z1Av[:, lo:NB_A, :], zo[:, :na, :])
                            nc.sync.dma_start(
                                z1Bv[:, 0:hi - NB_A, :], zo[:, na:nb, :])

            pref = int(os.environ.get("GCN_PREF", "1"))
            nl1 = int(os.environ.get("GCN_L1BLOCKS", str(BPC)))
            if phases >= 1:
                # phase 0 A-half (z1A rows), then prefetch the first L1-A
                # gather pieces so SWDGE ramps while phase 0 B computes.
                phase0_range(0, 200)
                if pref:
                    ensure_s2(1, "A", 0)
                    npieceA = (BPC * K1A * BLK + PIECE - 1) // PIECE
                    for pi in range(min(12, npieceA)):
                        ensure_g(1, "A", pi)
                phase0_range(208, NBLOCKS)

                for b in range(min(S0_BLOCKS, nl1)):
                    l1_block(b)
                if phases >= 2:
                    nc.gpsimd.collective_compute(
                        "AllGather", mybir.AluOpType.bypass,
                        replica_groups=[list(range(NCORES))],
                        ins=[z2in0.ap().opt()],
                        outs=[z2P0.ap().opt()])
                for b in range(S0_BLOCKS, min(BPC, nl1)):
                    l1_block(b)
                if phases >= 2:
                    nc.gpsimd.collective_compute(
                        "AllGather", mybir.AluOpType.bypass,
                        replica_groups=[list(range(NCORES))],
                        ins=[z2in1.ap().opt()],
                        outs=[z2P1.ap().opt()])
                if phases >= 3:
                    for b in range(BPC):
                        l2c_block(b)
                        l2d_block(b)

    nc.compile()
    return nc


# ---------------- entry point ----------------

def kernel(x, edge_index, W1, b1, W2, b2):
    x = np.asarray(x)
    edge_index = np.asarray(edge_index)
    in_maps, perms, Ks = _preprocess(x, edge_index, W1, b1, W2, b2)
    nc = _build_program(*Ks)

    trace = os.environ.get("GCN_TRACE", "0") == "1"
    if trace:
        trace = _install_trace_hook()
    res = run_bass_kernel_spmd(
        nc, in_maps, core_ids=list(range(NCORES)), trace=trace)
    LAST["exec_time_ns"] = res.exec_time_ns
    LAST["results"] = res

    out = np.empty((N_NODES, D_OUT), dtype=np.float32)
    for c in range(NCORES):
        oc = np.asarray(res.results[c]["out"], dtype=np.float32)
        out[c * RPC:(c + 1) * RPC] = oc[perms[c]]
    return out


# revision 56
# speedup vs baseline: 2.3168x; 1.1592x over previous
"""GCN (2-layer, PyG GCNConv semantics) on 8 Trainium2 NeuronCores.

Strategy (dst-sharded message passing):
  out = softmax( A @ relu(A @ (x W1) + b1) @ W2 + b2 ),  A = D^-1/2 (Adj+I) D^-1/2

  - Host: degrees/dinv, self-loops appended as ordinary edges, edges
    partitioned by destination core (6250 dst rows per core), each core's
    dst nodes permuted into 50 load-balanced blocks of 128.  Per-edge
    gather indices (int16) and one-hot segment-sum matrices (bf16, with
    dinv[dst] folded in; bias rows folded in as extra "edges") are
    precomputed on the host and streamed to the device.
  - Phase 0 (on-device, redundant per core): z1 = (dinv*x) @ W1 in bf16,
    stored to local HBM (the layer-1 gather table).
  - Phase 1: per-edge dma_gather of z1 rows (4 SWDGE queues round-robin,
    4096-row pieces); segment-sum via TensorE matmuls h += S^T @ G
    (S = one-hot with dinv[dst]); relu on ScalarE; z2 = dinv * (h @ W2).
  - AllGather of z2 (bf16, rows padded to 128 cols) across the 8 cores
    in two row-slices.
  - Phase 2: per-edge dma_gather of z2 rows (bf16 256B rows), segment-sum
    to output blocks, softmax, DMA out.

kernel(**inputs) -> np.ndarray is self-contained (shapes hardcoded).
"""

import os
import sys
import types

sys.path.insert(0, "/opt/trn_rl_repo")

import numpy as np
import ml_dtypes

from concourse import bass, mybir, bacc, tile
from concourse.bass_utils import run_bass_kernel_spmd

BF16 = ml_dtypes.bfloat16
FP8 = ml_dtypes.float8_e4m3   # matches mybir dt.float8e4 (TRN FP8_EXP4)
USE_FP8 = os.environ.get("GCN_FP8", "1") == "1"      # z1 table + L1 one-hots
USE_FP8S2 = os.environ.get("GCN_FP8S2", "1") == "1"  # L2 one-hots (mixed mm)
USE_FP8X = os.environ.get("GCN_FP8X", "1") == "1"    # x / W1 (phase 0)

# ---------------- problem constants (hardcoded) ----------------
N_NODES = 50000
D_IN, D_HID, D_OUT = 512, 256, 64
NCORES = 8
RPC = N_NODES // NCORES          # 6250 dst rows per core
BLK = 128
BPC = 50                         # blocks per core (spare slots for balancing)
RPAD = BPC * BLK                 # 6400
NPAD = ((N_NODES + BLK - 1) // BLK) * BLK   # 50048 (391 node blocks)
NBLOCKS = NPAD // BLK            # 391
SPLIT1 = 24960                   # L1 gather src split (block-aligned, int16-safe)
S0_ROWS = 3200                   # AG slice 0: perm positions [0, 3200) = 25 blocks
S1_ROWS = RPAD - S0_ROWS         # 3200: positions [3200, 6400) = 25 blocks
S0_BLOCKS = S0_ROWS // BLK       # 25
CPR = S0_ROWS + 16               # rows per AG contribution (16 = b2 bias pad)
Z2ROWS = NCORES * CPR            # 25728 rows per z2 table
PIECE = int(os.environ.get("GCN_PIECE", "1024"))   # slots per dma_gather
                                 # (>1024 overflows the SWDGE ring: hangs)
S2CH = 32                        # one-hot chunks per S2 stream DMA piece
NQ = 4                           # SWDGE queues (ucode max)

LAST = {}                        # test harness introspection


def _install_trace_hook():
    try:
        mod = types.ModuleType("antenv.axon_hooks")
        hook = [None]
        mod.set_axon_ntff_profile_hook = lambda h: hook.__setitem__(0, h)
        mod.get_axon_ntff_profile_hook = lambda: hook[0]
        sys.modules["antenv.axon_hooks"] = mod
        import antenv
        antenv.axon_hooks = mod
        from trn_agent_boot.trn_boot import _ntff_profile_via_ctypes
        mod.set_axon_ntff_profile_hook(
            _ntff_profile_via_ctypes("/opt/axon/libaxon_pjrt.so"))
        return True
    except Exception:
        return False


# ---------------- host-side preprocessing ----------------

def _pack_greedy(node_ids, cnts, block_ids, cap):
    """Greedy k-dim balanced packing of node_ids into block_ids (<=128 each).
    cnts: [ndim, RPC] per-node counts. Returns {node: block}."""
    nd = len(cnts)
    nb = len(block_ids)
    tot = sum(c[node_ids] for c in cnts)
    order = node_ids[np.argsort(-tot, kind="stable")]
    sums = np.zeros((nd, nb), dtype=np.float64)
    cnt = np.zeros(nb, dtype=np.int64)
    assign = {}
    big = 1e18
    for i in order:
        score = np.max([(sums[d] + cnts[d][i]) / cap for d in range(nd)], axis=0)
        score = score + (sums.sum(axis=0) + tot[0] * 0) * 1e-7
        score = np.where(cnt < BLK, score, big)
        j = int(np.argmin(score))
        assign[i] = j
        cnt[j] += 1
        for d in range(nd):
            sums[d, j] += cnts[d][i]
    # repair per dim
    members = {j: [i for i, jj in assign.items() if jj == j] for j in range(nb)}
    for d in range(nd):
        for _ in range(2000):
            j = int(np.argmax(sums[d]))
            if sums[d, j] <= cap:
                break
            ms = members[j]
            pos_m = [i for i in ms if cnts[d][i] > 0]
            if not pos_m:
                break
            mv = min(pos_m, key=lambda i: cnts[d][i])
            tgt = np.where(cnt < BLK, sums[d], big)
            tgt[j] = big
            jt = int(np.argmin(tgt))
            if tgt[jt] >= big:
                break
            assign[mv] = jt
            members[j].remove(mv)
            members[jt].append(mv)
            cnt[j] -= 1
            cnt[jt] += 1
            for dd in range(nd):
                sums[dd, j] -= cnts[dd][mv]
                sums[dd, jt] += cnts[dd][mv]
    return assign


def _positions_from_assign(assign, block_ids):
    pos = {}
    slot = {j: 0 for j in block_ids}
    for i in sorted(assign):
        j = assign[i]
        pos[i] = j * BLK + slot[j]
        slot[j] += 1
    return pos


def _pack_blocks(cntA, cntB, cap=1148):
    nodes = np.arange(RPC)
    assign = _pack_greedy(nodes, [cntA, cntB], list(range(BPC)), cap)
    posd = _positions_from_assign(assign, list(range(BPC)))
    pos = np.empty(RPC, dtype=np.int64)
    for i in range(RPC):
        pos[i] = posd[i]
    return pos


def _pack_blocks4(cntA, cntB, cntC, cntD, half0_nodes, cap=1148):
    """Second pass: rebalance within halves on 4 dims."""
    pos = np.empty(RPC, dtype=np.int64)
    all_nodes = np.arange(RPC)
    h0 = half0_nodes
    h1 = all_nodes[~np.isin(all_nodes, h0)]
    for nodes, blocks in ((h0, list(range(S0_BLOCKS))),
                          (h1, list(range(S0_BLOCKS, BPC)))):
        assign = _pack_greedy(nodes, [cntA, cntB, cntC, cntD], blocks, cap)
        # blocks list indexes into _pack_greedy's local 0..nb-1 space
        posd = {}
        slot = {j: 0 for j in range(len(blocks))}
        for i in sorted(assign):
            j = assign[i]
            posd[i] = blocks[j] * BLK + slot[j]
            slot[j] += 1
        for i in nodes:
            pos[i] = posd[i]
    return pos


def _build_stream(e_pos, e_idx16, e_dd, K, bias_idx=None, sdt=BF16):
    """Returns (idx_wrapped [128, SL/16] i16, s2 [128, nch*128] sdt).
    e_dd: per-edge weight folded into the one-hot matrix (dinv[dst]).
    bias_idx: if set, one extra slot per block gathers this row and adds it
    (weight 1.0) to every dst position of the block (bias fold-in)."""
    nch = BPC * K
    SL = nch * BLK
    blk = e_pos // BLK
    # block-major, src-sorted within block: consecutive gather descriptors
    # hit ascending table rows (HBM locality)
    o = np.lexsort((e_idx16.astype(np.int32), blk))
    blk_s = blk[o]
    e_pos = e_pos[o]
    e_idx16 = e_idx16[o]
    dd = e_dd[o] if e_dd is not None else np.ones(len(o), np.float32)
    counts = np.bincount(blk_s, minlength=BPC)
    cap = K * BLK - (1 if bias_idx is not None else 0)
    assert counts.max() <= cap, (counts.max(), cap)
    starts = np.concatenate([[0], np.cumsum(counts)[:-1]])
    within = np.arange(len(blk_s)) - np.repeat(starts, counts)
    slot = blk_s * (K * BLK) + within

    idx_full = np.zeros(SL, dtype=np.int16)
    idx_full[slot] = e_idx16
    s2 = np.zeros((128, nch, 128), dtype=sdt)
    s2[slot % BLK, slot // BLK, (e_pos % BLK)] = dd.astype(sdt)
    if bias_idx is not None:
        for b in range(BPC):
            fs = b * (K * BLK) + counts[b]     # first free slot of block b
            idx_full[fs] = bias_idx
            s2[fs % BLK, fs // BLK, :] = np.ones(128, dtype=sdt)
    idx_w = np.tile(idx_full.reshape(SL // 16, 16).T, (8, 1)).copy()
    return idx_w, s2.reshape(128, nch * 128)


def _preprocess(x, edge_index, W1, b1, W2, b2):
    src = np.asarray(edge_index[0], dtype=np.int64)
    dst = np.asarray(edge_index[1], dtype=np.int64)
    loops = np.arange(N_NODES, dtype=np.int64)
    src_all = np.concatenate([src, loops])
    dst_all = np.concatenate([dst, loops])
    deg = np.bincount(dst_all, minlength=N_NODES).astype(np.float32)
    dinv = np.where(deg > 0, 1.0 / np.sqrt(deg), 0.0).astype(np.float32)

    core_of = dst_all // RPC

    perms = []
    core_edges = []
    cnts_ab = []
    for c in range(NCORES):
        m = core_of == c
        s_c = src_all[m]
        d_loc = (dst_all[m] - c * RPC).astype(np.int64)
        cntA = np.bincount(d_loc[s_c < SPLIT1], minlength=RPC)
        cntB = np.bincount(d_loc[s_c >= SPLIT1], minlength=RPC)
        perms.append(_pack_blocks(cntA, cntB))
        core_edges.append((s_c, d_loc))
        cnts_ab.append((cntA, cntB))

    permpos_global = np.empty(N_NODES, dtype=np.int64)
    for c in range(NCORES):
        permpos_global[c * RPC:(c + 1) * RPC] = perms[c]

    # pass 2: rebalance within halves, also evening C/D (src-half) counts
    half_global = permpos_global < S0_ROWS
    perms2 = []
    for c in range(NCORES):
        s_c, d_loc = core_edges[c]
        cntA, cntB = cnts_ab[c]
        hsrc = half_global[s_c]
        cntC = np.bincount(d_loc[hsrc], minlength=RPC)
        cntD = np.bincount(d_loc[~hsrc], minlength=RPC)
        half0_nodes = np.where(perms[c] < S0_ROWS)[0]
        perms2.append(_pack_blocks4(cntA, cntB, cntC, cntD, half0_nodes))
    perms = perms2
    for c in range(NCORES):
        permpos_global[c * RPC:(c + 1) * RPC] = perms[c]

    def seg_K(e_pos, extra=0):
        counts = np.bincount(e_pos // BLK, minlength=BPC)
        return int(np.ceil((counts.max() + extra) / BLK))

    K1A = K1B = K2C = K2D = 1
    meta = []
    for c in range(NCORES):
        s_c, d_loc = core_edges[c]
        pos_d = perms[c][d_loc]
        mA = s_c < SPLIT1
        src_r = s_c // RPC
        src_pos = permpos_global[s_c]   # core-local position (0..RPAD-1)
        mC = src_pos < S0_ROWS
        K1A = max(K1A, seg_K(pos_d[mA], 1))
        K1B = max(K1B, seg_K(pos_d[~mA]))
        K2C = max(K2C, seg_K(pos_d[mC], 1))
        K2D = max(K2D, seg_K(pos_d[~mC]))
        meta.append((s_c, d_loc, pos_d, mA, mC, src_r, src_pos))

    in_maps = []
    xdt = FP8 if USE_FP8X else BF16
    xs = (np.asarray(x, np.float32) * dinv[:, None])
    xT = np.zeros((D_IN, NPAD), dtype=xdt)
    xT[:, :N_NODES] = xs.T.astype(xdt)
    w1b = np.asarray(W1, np.float32).astype(xdt)
    w2b = np.asarray(W2, np.float32).astype(BF16)
    ident = np.eye(128, dtype=np.float32).astype(BF16)
    # bias rows: b1 as a gatherable z1-table row, b2 as a z2-table row
    b1row = np.zeros((128, D_HID), dtype=FP8 if USE_FP8 else BF16)
    b1row[0, :] = np.asarray(b1, np.float32).astype(b1row.dtype)
    b2row = np.zeros((16, 128), dtype=BF16)
    b2row[0, :D_OUT] = np.asarray(b2, np.float32).astype(BF16)

    real = padded = 0
    for c in range(NCORES):
        s_c, d_loc, pos_d, mA, mC, src_r, src_pos = meta[c]
        dd = dinv[d_loc + c * RPC]    # dinv[dst] per edge
        s1dt = FP8 if USE_FP8 else BF16
        s2dt = FP8 if USE_FP8S2 else BF16
        # z1 tables are partition-major: node v -> row (v%128)*NB + v//128
        NB_A = SPLIT1 // BLK
        NB_B = (NPAD - SPLIT1) // BLK
        vA = s_c[mA]
        idxA = ((vA % BLK) * NB_A + vA // BLK).astype(np.int16)
        vB = s_c[~mA] - SPLIT1
        idxB = ((vB % BLK) * NB_B + vB // BLK).astype(np.int16)
        i1a, s2a = _build_stream(pos_d[mA], idxA,
                                 dd[mA], K1A, bias_idx=SPLIT1, sdt=s1dt)
        i1b, s2b = _build_stream(pos_d[~mA], idxB,
                                 dd[~mA], K1B, sdt=s1dt)
        idxC = (src_r * CPR + src_pos).astype(np.int16)
        idxD = (src_r * CPR + (src_pos - S0_ROWS)).astype(np.int16)
        i2c, s2c = _build_stream(pos_d[mC], idxC[mC], dd[mC], K2C,
                                 bias_idx=S0_ROWS, sdt=s2dt)
        i2d, s2d = _build_stream(pos_d[~mC], idxD[~mC], dd[~mC], K2D,
                                 sdt=s2dt)

        dinvb = np.zeros((BLK, BPC), dtype=np.float32)
        nodes_at = np.full(RPAD, -1, dtype=np.int64)
        nodes_at[perms[c]] = np.arange(RPC)
        valid = nodes_at >= 0
        dv = np.zeros(RPAD, np.float32)
        dv[valid] = dinv[nodes_at[valid] + c * RPC]
        dinvb[:, :] = dv.reshape(BPC, BLK).T

        in_maps.append({
            "xT": xT, "w1": w1b, "w2": w2b, "ident": ident,
            "b1row": b1row, "b2row": b2row,
            "dinvb": dinvb,
            "i1a": i1a, "s2a": s2a, "i1b": i1b, "s2b": s2b,
            "i2c": i2c, "s2c": s2c, "i2d": i2d, "s2d": s2d,
        })
        real += len(s_c)
        padded += BLK * BPC * (K1A + K1B)

    LAST["K"] = (K1A, K1B, K2C, K2D)
    LAST["pad_frac"] = padded / real - 1.0
    return in_maps, perms, (K1A, K1B, K2C, K2D)


# ---------------- device program ----------------

def _build_program(K1A, K1B, K2C, K2D):
    dt = mybir.dt
    Z1DT = dt.float8e4 if USE_FP8 else dt.bfloat16
    S2DT = dt.float8e4 if USE_FP8S2 else dt.bfloat16
    XDT = dt.float8e4 if USE_FP8X else dt.bfloat16
    phases = int(os.environ.get("GCN_PHASES", "3"))
    nc = bacc.Bacc(None, target_bir_lowering=False, debug=False,
                   num_devices=NCORES, num_swdge_queues=NQ,
                   dynamic_dma_scratch_size=int(
                       os.environ.get("GCN_SCRATCH", "16384")))

    xT = nc.dram_tensor("xT", [D_IN, NPAD], XDT, kind="ExternalInput")
    w1 = nc.dram_tensor("w1", [D_IN, D_HID], XDT, kind="ExternalInput")
    w2 = nc.dram_tensor("w2", [D_HID, D_OUT], dt.bfloat16, kind="ExternalInput")
    ident = nc.dram_tensor("ident", [128, 128], dt.bfloat16, kind="ExternalInput")
    b1row = nc.dram_tensor("b1row", [128, D_HID], Z1DT,
                           kind="ExternalInput")
    b2row = nc.dram_tensor("b2row", [16, 128], dt.bfloat16,
                           kind="ExternalInput")
    dinvb = nc.dram_tensor("dinvb", [128, BPC], dt.float32, kind="ExternalInput")

    def idx_t(name, K):
        return nc.dram_tensor(name, [128, BPC * K * BLK // 16], dt.int16,
                              kind="ExternalInput")

    def s2_t(name, K, sdt):
        return nc.dram_tensor(name, [128, BPC * K * BLK], sdt,
                              kind="ExternalInput")

    i1a, s2a = idx_t("i1a", K1A), s2_t("s2a", K1A, Z1DT)
    i1b, s2b = idx_t("i1b", K1B), s2_t("s2b", K1B, Z1DT)
    i2c, s2c = idx_t("i2c", K2C), s2_t("s2c", K2C, S2DT)
    i2d, s2d = idx_t("i2d", K2D), s2_t("s2d", K2D, S2DT)

    out = nc.dram_tensor("out", [RPAD, D_OUT], dt.float32, kind="ExternalOutput")

    z1A = nc.dram_tensor("z1A", [SPLIT1 + 128, D_HID], Z1DT)
    z1B = nc.dram_tensor("z1B", [NPAD - SPLIT1, D_HID], Z1DT)
    z2in0 = nc.dram_tensor("z2in0", [CPR, 128], dt.bfloat16)
    z2in1 = nc.dram_tensor("z2in1", [CPR, 128], dt.bfloat16)
    z2P0 = nc.dram_tensor("z2P0", [Z2ROWS, 128], dt.bfloat16,
                          addr_space="Shared")
    z2P1 = nc.dram_tensor("z2P1", [Z2ROWS, 128], dt.bfloat16,
                          addr_space="Shared")

    qctr = [0]

    def next_q():
        q = qctr[0] % NQ
        qctr[0] += 1
        return q

    with tile.TileContext(nc) as tc:
        with tc.tile_pool(name="consts", bufs=1) as cp, \
             tc.tile_pool(name="ph0x", bufs=2) as xp, \
             tc.tile_pool(name="ph0o", bufs=3) as op0, \
             tc.tile_pool(name="gp", bufs=4) as gp, \
             tc.tile_pool(name="csp", bufs=BPC + 1) as csp, \
             tc.tile_pool(name="s2p", bufs=2) as s2p, \
             tc.tile_pool(name="hp", bufs=2) as hp, \
             tc.tile_pool(name="zp", bufs=3) as zp, \
             tc.tile_pool(name="smp", bufs=8) as smp, \
             tc.tile_pool(name="psAcc", bufs=3, space="PSUM") as psAcc, \
             tc.tile_pool(name="psMisc", bufs=1, space="PSUM") as psMisc, \
             tc.tile_pool(name="psO", bufs=3, space="PSUM") as psO:
            w1t = cp.tile([128, 4, D_HID], XDT)
            nc.sync.dma_start(
                w1t[:], w1.ap().rearrange("(k p) n -> p k n", p=128))
            w2t = cp.tile([128, 2, D_OUT], dt.bfloat16)
            nc.sync.dma_start(
                w2t[:], w2.ap().rearrange("(k p) n -> p k n", p=128))
            idt = cp.tile([128, 128], dt.bfloat16)
            nc.sync.dma_start(idt[:], ident[:, :])
            dvt = cp.tile([128, BPC], dt.float32)
            nc.sync.dma_start(dvt[:], dinvb[:, :])
            # bias rows into the gather tables / AG contributions
            nc.sync.dma_start(z1A.ap()[SPLIT1:SPLIT1 + 128, :], b1row.ap()[:, :])
            nc.sync.dma_start(z2in0.ap()[S0_ROWS:CPR, :], b2row.ap()[:, :])
            nc.sync.dma_start(z2in1.ap()[S0_ROWS:CPR, :], b2row.ap()[:, :])
            it1a = cp.tile([128, BPC * K1A * BLK // 16], dt.int16)
            nc.scalar.dma_start(it1a[:], i1a[:, :])
            it1b = cp.tile([128, BPC * K1B * BLK // 16], dt.int16)
            nc.scalar.dma_start(it1b[:], i1b[:, :])
            it2c = cp.tile([128, BPC * K2C * BLK // 16], dt.int16)
            nc.scalar.dma_start(it2c[:], i2c[:, :])
            it2d = cp.tile([128, BPC * K2D * BLK // 16], dt.int16)
            nc.scalar.dma_start(it2d[:], i2d[:, :])
            itabs = {(1, "A"): it1a, (1, "B"): it1b,
                     (2, "C"): it2c, (2, "D"): it2d}

            # ---------------- phases 1+2 stream tables ----------------
            seg1 = {
                "A": (K1A, i1a, s2a, z1A.ap()[:, :]),
                "B": (K1B, i1b, s2b, z1B.ap()[:, :]),
            }
            seg2 = {
                "C": (K2C, i2c, s2c, z2P0.ap()[:, :]),
                "D": (K2D, i2d, s2d, z2P1.ap()[:, :]),
            }
            gtiles = {}
            s2tiles = {}

            def ensure_g(layer, s, pi):
                key = (layer, s, pi)
                if key in gtiles:
                    return gtiles[key]
                K, idrm, s2drm, zview = (seg1 if layer == 1 else seg2)[s]
                felem = D_HID if layer == 1 else 128
                gdt = Z1DT if layer == 1 else dt.bfloat16
                SL = BPC * K * BLK
                n = min(PIECE, SL - pi * PIECE)
                off = pi * (PIECE // 16)
                it = itabs[(layer, s)]
                gt = gp.tile([128, PIECE // 128, felem], gdt,
                             tag=f"g{layer}{s}",
                             bufs=(12 if layer == 1 else 6))
                nc.gpsimd.dma_gather(
                    gt[:, :n // 128, :], zview, it[:, off:off + n // 16],
                    n, n, felem, queue_num=next_q())
                gtiles[key] = gt
                return gt

            def ensure_s2(layer, s, pi):
                key = (layer, s, pi)
                if key in s2tiles:
                    return s2tiles[key]
                K, idrm, s2drm, zview = (seg1 if layer == 1 else seg2)[s]
                sdt = Z1DT if layer == 1 else S2DT
                nch = BPC * K
                n = min(S2CH, nch - pi * S2CH)
                st = s2p.tile([128, S2CH, 128], sdt, tag=f"s{layer}{s}")
                nc.scalar.dma_start(
                    st[:, :n, :],
                    s2drm.ap()[:, pi * S2CH * 128:(pi * S2CH + n) * 128]
                    .rearrange("p (n c) -> p n c", c=128))
                s2tiles[key] = st
                return st

            PCH = PIECE // BLK       # gather chunks per piece

            def l1_block(b):
                # psum accumulates sum_e dinv[d]*dinv[s]*z1[s] + b1 directly
                hps = psAcc.tile([128, D_HID], dt.float32, tag="acc")
                for s in ("A", "B"):
                    K = seg1[s][0]
                    k = 0
                    while k < K:
                        ci = b * K + k
                        gpi, gpos = divmod(ci * BLK, PIECE)
                        spi, spos = divmod(ci, S2CH)
                        cp_ = gpos // BLK
                        gt = ensure_g(1, s, gpi)
                        st = ensure_s2(1, s, spi)
                        pair = (USE_FP8 and k + 1 < K and cp_ + 1 < PCH
                                and spos + 1 < S2CH)
                        start = (s == "A" and k == 0)
                        if pair:
                            stop = (s == "B" and k + 2 == K)
                            nc.tensor.matmul(
                                hps[:],
                                st[:, spos:spos + 2, :],
                                gt[:, cp_:cp_ + 2, :],
                                start=start, stop=stop,
                                perf_mode=mybir.MatmulPerfMode.DoubleRow)
                            k += 2
                        else:
                            stop = (s == "B" and k + 1 == K)
                            nc.tensor.matmul(
                                hps[:],
                                st[:, spos, :],
                                gt[:, cp_, :],
                                start=start, stop=stop)
                            k += 1
                hr = hp.tile([128, D_HID], dt.bfloat16, tag="hr")
                nc.scalar.activation(
                    hr[:], hps[:], mybir.ActivationFunctionType.Relu)
                hT = hp.tile([128, 2, 128], dt.bfloat16, tag="hT")
                for h in range(2):
                    tps = psMisc.tile([128, 128], dt.bfloat16, tag="tps")
                    nc.tensor.transpose(
                        tps[:], hr[:, h * 128:(h + 1) * 128], idt[:])
                    nc.scalar.copy(hT[:, h, :], tps[:])
                zps = psMisc.tile([128, D_OUT], dt.float32, tag="zps")
                for h in range(2):
                    nc.tensor.matmul(
                        zps[:], hT[:, h, :], w2t[:, h, :],
                        start=(h == 0), stop=(h == 1))
                z2s = zp.tile([128, 128], dt.bfloat16, tag="z2s")
                nc.scalar.activation(
                    z2s[:, :D_OUT], zps[:],
                    mybir.ActivationFunctionType.Copy, scale=dvt[:, b:b + 1])
                if b < S0_BLOCKS:
                    nc.sync.dma_start(
                        z2in0.ap()[b * BLK:(b + 1) * BLK, :], z2s[:])
                else:
                    bb = b - S0_BLOCKS
                    nc.sync.dma_start(
                        z2in1.ap()[bb * BLK:(bb + 1) * BLK, :], z2s[:])

            cstash = {}

            def l2cd_block(b, s):
                ops = psO.tile([128, D_OUT], dt.float32, tag="ops")
                K = seg2[s][0]
                for k in range(K):
                    ci = b * K + k
                    gpi, gpos = divmod(ci * BLK, PIECE)
                    spi, spos = divmod(ci, S2CH)
                    gt = ensure_g(2, s, gpi)
                    st = ensure_s2(2, s, spi)
                    nc.tensor.matmul(
                        ops[:],
                        st[:, spos, :],
                        gt[:, (gpos // BLK), :D_OUT],
                        start=(k == 0), stop=(k == K - 1))
                return ops

            def l2c_block(b):
                ops = l2cd_block(b, "C")
                cs = csp.tile([128, D_OUT], dt.float32, tag="cs")
                nc.scalar.copy(cs[:], ops[:])
                cstash[b] = cs

            def l2d_block(b):
                ops = l2cd_block(b, "D")
                ds = smp.tile([128, D_OUT], dt.float32, tag="ds")
                nc.scalar.copy(ds[:], ops[:])     # PSUM->SBUF (DVE hates PSUM)
                t2 = smp.tile([128, D_OUT], dt.float32, tag="t2")
                nc.vector.tensor_tensor(
                    t2[:], ds[:], cstash[b][:], op=mybir.AluOpType.add)
                nm = smp.tile([128, 1], dt.float32, tag="nm")
                nc.vector.reduce_max(
                    nm[:], t2[:], axis=mybir.AxisListType.X, negate=True)
                ex = smp.tile([128, D_OUT], dt.float32, tag="ex")
                sm = smp.tile([128, 1], dt.float32, tag="sm")
                nc.scalar.activation(
                    ex[:], t2[:], mybir.ActivationFunctionType.Exp,
                    bias=nm[:], accum_out=sm[:])
                rc = smp.tile([128, 1], dt.float32, tag="rc")
                nc.vector.reciprocal(rc[:], sm[:])
                ot = smp.tile([128, D_OUT], dt.float32, tag="ot")
                nc.vector.tensor_scalar(
                    ot[:], ex[:], rc[:], None, op0=mybir.AluOpType.mult)
                nc.sync.dma_start(out.ap()[b * BLK:(b + 1) * BLK, :], ot[:])

            # ---------------- phase 0: z1 = xT^T @ W1 (A half then B half) ---
            # partition-major tables: row = p*NB + n, so each partition's
            # rows are contiguous -> few, fat write descriptors
            z1Av = z1A.ap()[0:SPLIT1, :].rearrange("(p n) f -> p n f", p=128)
            z1Bv = z1B.ap().rearrange("(p n) f -> p n f", p=128)
            NB_A = SPLIT1 // BLK
            GB = 7
            GRP = 32

            def phase0_range(glo, ghi):
                for g0 in range(glo, ghi, GRP):
                    gb = min(GRP, NBLOCKS - g0)
                    if gb <= 0:
                        break
                    xg = xp.tile([128, 4, GRP * BLK], XDT, tag="xg")
                    nc.sync.dma_start(
                        xg[:, :, :gb * BLK],
                        xT.ap().rearrange("(k p) n -> p k n", p=128)
                        [:, :, g0 * BLK:(g0 + gb) * BLK])
                    for b0 in range(0, gb, GB):
                        nb = min(GB, gb - b0)
                        zo = op0.tile([128, GB, D_HID], Z1DT, tag="zo")
                        for i in range(nb):
                            ps = psAcc.tile([128, D_HID], dt.float32, tag="acc")
                            col = (b0 + i) * BLK
                            if USE_FP8X:
                                for k in (0, 2):
                                    nc.tensor.matmul(
                                        ps[:],
                                        xg[:, k:k + 2, col:col + BLK],
                                        w1t[:, k:k + 2, :],
                                        start=(k == 0), stop=(k == 2),
                                        perf_mode=mybir.MatmulPerfMode
                                        .DoubleRow)
                            else:
                                for k in range(4):
                                    nc.tensor.matmul(
                                        ps[:],
                                        xg[:, k, col:col + BLK],
                                        w1t[:, k, :],
                                        start=(k == 0), stop=(k == 3))
                            if (b0 + i) % 2 == 0:
                                nc.vector.tensor_copy(zo[:, i, :], ps[:])
                            else:
                                nc.scalar.copy(zo[:, i, :], ps[:])
                        lo, hi = g0 + b0, g0 + b0 + nb
                        if hi <= NB_A:
                            nc.sync.dma_start(z1Av[:, lo:hi, :], zo[:, :nb, :])
                        elif lo >= NB_A:
                            nc.sync.dma_start(
                                z1Bv[:, lo - NB_A:hi - NB_A, :], zo[:, :nb, :])
                        else:
                            na = NB_A - lo
                            nc.sync.dma_start(z1Av[:, lo:NB_A, :], zo[:, :na, :])
                            nc.sync.dma_start(
                                z1Bv[:, 0:hi - NB_A, :], zo[:, na:nb, :])

            pref = int(os.environ.get("GCN_PREF", "1"))
            nl1 = int(os.environ.get("GCN_L1BLOCKS", str(BPC)))
            if phases >= 1:
                # phase 0 A-half (z1A rows), then prefetch the first L1-A
                # gather pieces so SWDGE ramps while phase 0 B computes.
                phase0_range(0, 195)
                if pref:
                    ensure_s2(1, "A", 0)
                    npieceA = (BPC * K1A * BLK + PIECE - 1) // PIECE
                    for pi in range(min(12, npieceA)):
                        ensure_g(1, "A", pi)
                phase0_range(224, NBLOCKS)

                for b in range(min(S0_BLOCKS, nl1)):
                    l1_block(b)
                if phases >= 2:
                    nc.gpsimd.collective_compute(
                        "AllGather", mybir.AluOpType.bypass,
                        replica_groups=[list(range(NCORES))],
                        ins=[z2in0.ap().opt()],
                        outs=[z2P0.ap().opt()])
                for b in range(S0_BLOCKS, min(BPC, nl1)):
                    l1_block(b)
                if phases >= 2:
                    nc.gpsimd.collective_compute(
                        "AllGather", mybir.AluOpType.bypass,
                        replica_groups=[list(range(NCORES))],
                        ins=[z2in1.ap().opt()],
                        outs=[z2P1.ap().opt()])
                if phases >= 3:
                    for b in range(BPC):
                        l2c_block(b)
                        l2d_block(b)

    nc.compile()
    return nc


# ---------------- entry point ----------------

def kernel(x, edge_index, W1, b1, W2, b2):
    x = np.asarray(x)
    edge_index = np.asarray(edge_index)
    in_maps, perms, Ks = _preprocess(x, edge_index, W1, b1, W2, b2)
    nc = _build_program(*Ks)

    trace = os.environ.get("GCN_TRACE", "0") == "1"
    if trace:
        trace = _install_trace_hook()
    res = run_bass_kernel_spmd(
        nc, in_maps, core_ids=list(range(NCORES)), trace=trace)
    LAST["exec_time_ns"] = res.exec_time_ns
    LAST["results"] = res

    out = np.empty((N_NODES, D_OUT), dtype=np.float32)
    for c in range(NCORES):
        oc = np.asarray(res.results[c]["out"], dtype=np.float32)
        out[c * RPC:(c + 1) * RPC] = oc[perms[c]]
    return out
